# revision 1
# baseline (speedup 1.0000x reference)
"""Trainium2 Bass kernel for DecoderWithAttention (bidirectional 2-layer LSTM +
additive attention + gated fc), data-parallel over batch across 8 NeuronCores.

Shapes (hardcoded): encoder_out (64, 512, 16, 16), T=16, D=A=512, V=5000.
Per core: 8 batches, full network, weights replicated (no collectives available
under this axon terminal, so each core is fully independent).

Key layout decisions (per core):
  - All matmuls weight-stationary: matmul(out, lhsT, rhs): out = lhsT^T @ rhs.
  - LSTM gates PSUM: [128 part = gate%128, cols = (gate_chunk 16, batch 8)].
  - Input projections for all 16 steps batched (N=128); only Whh per step.
  - Hidden stores H*: [128, dch(4), t(16), b(8)] bf16, logical-t order (the
    reverse cells index t=15-s at compile time, so no data reversal anywhere).
  - Attention transposed (A on partitions). sum_a w_a*relu(v_a) uses the
    signed w directly as the PE reduction rhs (relu(s*x)=s*relu(x) for s>0
    means no |w| prefold is needed anywhere). Softmax over p via PE ones-sum
    in [p, (b,t)] layout, no max subtraction (|score| bounded), bfull dropped
    (softmax shift invariance).
  - att2 (Wdec@h2 + bea) is computed per-timestep inside the L2 loop as each
    h2[t] completes; a slice of the relu/score work is interleaved there too.
  - fc factorized around the gate: out = g0*(hid^T Wfc_h + bfc)
    + g1*(awe^T Wfc_a + bfc) (g0+g1==1). The hidden part (2/3 of Wfc bytes
    and flops) streams + computes during the attention phase; gates are
    per-(b,t) scalars applied on psum partitions post-hoc.
  - fc output psum is [(b,t) parts, v cols] so writebacks are v-contiguous
    2KB DMA runs.
  - gate softmax(2) == sigmoid(logit diff), Wg[0]-Wg[1] folded host-side.
  - Mean over H folded into Wih1 (1/16); bih+bhh folded host-side.
"""

import numpy as np
import ml_dtypes

BF = ml_dtypes.bfloat16
B, E, HH, WW = 64, 512, 16, 16
T = WW          # 16 timesteps
PP = HH * WW    # 256 attention positions
D = 512
A = 512
V = 5000
G = 4 * D
NB = 8          # batches per core
NCORES = 8
F = 2 * D + E   # 1536
VCH = (V + 127) // 128  # 40 (last chunk has 8)
NVB = 10        # fc v-blocks of 512

_prog_cache = {}


def _build_program():
    import concourse.bass as bass
    import concourse.bacc as bacc
    import concourse.mybir as mybir
    import concourse.tile as tile

    dt = mybir.dt
    AF = mybir.ActivationFunctionType
    ALU = mybir.AluOpType

    nc = bacc.Bacc("TRN2", target_bir_lowering=False, debug=False,
                   num_devices=NCORES, dynamic_dma_scratch_size=2048)

    def din(name, shape, d=dt.bfloat16):
        return nc.dram_tensor(name, shape, d, kind="ExternalInput")

    enc_ep = din("enc_ep", [NB, E, PP])          # [b, e, p]
    enc_pe = din("enc_pe", [NB, PP, E])          # [b, p, e]
    wih1 = {0: din("wih1f", [E, G]), 1: din("wih1r", [E, G])}
    whh1 = {0: din("whh1f", [D, G]), 1: din("whh1r", [D, G])}
    wih2 = {0: din("wih2f", [2 * D, G]), 1: din("wih2r", [2 * D, G])}
    whh2 = {0: din("whh2f", [D, G]), 1: din("whh2r", [D, G])}
    b1 = {0: din("b1f", [G]), 1: din("b1r", [G])}
    b2 = {0: din("b2f", [G]), 1: din("b2r", [G])}
    wencT = din("wencT", [E, A])
    wdecT = din("wdecT", [2 * D, A])
    beab = din("beab", [A])                      # benc + bdec, bf16
    wfullb = din("wfullb", [A])                  # Wfull[0] bf16 (signed)
    wdiffT = din("wdiffT", [F])                  # Wg[0]-Wg[1] bf16
    bdiffb = din("bdiffb", [1, 1])               # bg0-bg1 bf16
    eye128 = din("eye128", [128, 128])
    wfcT = din("wfcT", [F, V])
    bfcp = din("bfcp", [VCH * 128])
    out_t = nc.dram_tensor("out", [NB, T, V], dt.float32, kind="ExternalOutput")

    with tile.TileContext(nc) as tc:
        with (
            tc.tile_pool(name="const", bufs=1) as const,
            tc.tile_pool(name="wbig", bufs=3) as wbig,
            tc.tile_pool(name="work", bufs=4) as work,
            tc.tile_pool(name="rwp", bufs=12) as rwp,
            tc.tile_pool(name="wfcp", bufs=2) as wfcp,
            tc.tile_pool(name="outp", bufs=2) as outp,
            tc.tile_pool(name="ps_g", bufs=3, space="PSUM") as ps_g,
            tc.tile_pool(name="ps_mm", bufs=3, space="PSUM") as ps_mm,
            tc.tile_pool(name="ps_sc", bufs=1, space="PSUM") as ps_sc,
        ):
            dma = nc.sync.dma_start

            # ---------------- persistent SBUF ----------------
            # DMA issue order is the SP program order: tiny tiles the first
            # recurrence steps depend on (eye for the identity-add matmuls,
            # b1 rows for closing the projection psums) go absolutely first,
            # then enc_ep (feats), then layer-1 weights; all later-phase
            # tensors queue after those.
            eye_sb = const.tile([128, 128], dt.bfloat16)
            dma(out=eye_sb[:], in_=eye128[:])
            b1row, b2row = {}, {}
            for d_ in (0, 1):
                b1row[d_] = const.tile([1, G], dt.bfloat16, tag=f"b1r_{d_}",
                                       bufs=1, name=f"b1row{d_}")
                dma(out=b1row[d_][:], in_=b1[d_][:])
            enc_ep_sb = const.tile([128, NB, 4, PP], dt.bfloat16)   # (b,ech,p)
            for bh in (0, 1):
                dma(out=enc_ep_sb[:, 4 * bh:4 * bh + 4, :, :],
                    in_=enc_ep[4 * bh:4 * bh + 4]
                    .rearrange("b (ec ep) p -> ep b ec p", ep=128))

            feats = const.tile([128, 4, NB, T], dt.bfloat16)  # (ech, b, w)
            # Xp2 reuses the Xp1 slots (same size/tag; the tile framework
            # stalls the second writer until layer-1 readers are done)
            Xp1 = {d_: const.tile([128, 16, NB, T], dt.bfloat16,
                                  tag=f"xp_{d_}", bufs=1, name=f"Xp1_{d_}")
                   for d_ in (0, 1)}                          # (gch, b, w)
            H1 = {d_: const.tile([128, 4, T, NB], dt.bfloat16, tag=f"h1_{d_}", name=f"H1_{d_}")
                  for d_ in (0, 1)}                           # (dch, t, b)
            H2 = {d_: const.tile([128, 4, T, NB], dt.bfloat16, tag=f"h2_{d_}", name=f"H2_{d_}")
                  for d_ in (0, 1)}
            att1w = const.tile([128, NB, 4, PP], dt.bfloat16)  # (b, ach, p)
            att2pb = const.tile([128, 4, 128], dt.float32)     # (ach, (b,t))
            att2pb_r = att2pb[:].rearrange("p a (b t) -> p a t b", t=T)
            alphaT = const.tile([128, 2, 128], dt.bfloat16)    # (pch, (b,t))
            aweT = const.tile([128, 4, 128], dt.bfloat16)      # (ech, (b,t))
            E_sb = const.tile([128, 2, 128], dt.bfloat16)
            Hw = const.tile([128, 8, 128], dt.bfloat16)        # (kch, (b,t))
            U_sb = [const.tile([128, 512], dt.bfloat16, tag=f"u{vb}",
                               name=f"U{vb}") for vb in range(NVB)]
            g01T = const.tile([128, 2], dt.float32)
            recip_sb = const.tile([1, 128], dt.float32)
            ones1_sb = const.tile([1, 128], dt.float32)
            nc.vector.memset(ones1_sb[:], 1.0)
            ones1b_sb = const.tile([1, 128], dt.bfloat16)
            nc.vector.memset(ones1b_sb[:], 1.0)

            # ---------- LSTM weights (stream through shared 4-slot pool) ----
            def load_w(dram, kchunks):
                # list of [128, 4, G] tiles (each 4 k-chunks) sharing one tag
                tiles = []
                for blk in range(kchunks // 4):
                    t_ = wbig.tile([128, 4, G], dt.bfloat16, tag="w", bufs=4,
                                   name="wtile")
                    dma(out=t_[:],
                        in_=dram[:].rearrange("(kc kp) g -> kp kc g", kp=128)
                        [:, blk * 4:(blk + 1) * 4, :])
                    tiles.append(t_)
                return tiles

            # layer-1 weights queue right behind enc_ep; everything the
            # later phases need comes after so the head stays short
            wih1_sb = {d_: load_w(wih1[d_], 4) for d_ in (0, 1)}
            whh1_sb = {d_: load_w(whh1[d_], 4) for d_ in (0, 1)}
            whh1_view = {d_: whh1_sb[d_][0] for d_ in (0, 1)}

            for d_ in (0, 1):
                b2row[d_] = const.tile([1, G], dt.bfloat16, tag=f"b2r_{d_}",
                                       name=f"b2row{d_}")
                dma(out=b2row[d_][:], in_=b2[d_][:])

            enc_pe_sb = const.tile([128, NB, 2, E], dt.bfloat16)    # (b,pch,e)
            dma(out=enc_pe_sb[:],
                in_=enc_pe[:].rearrange("b (pc pp) e -> pp b pc e", pp=128))
            wencT_sb = const.tile([128, 4, A], dt.bfloat16)   # (ech, a)
            dma(out=wencT_sb[:],
                in_=wencT[:].rearrange("(ec ep) a -> ep ec a", ep=128))
            wdecT_sb = const.tile([128, 8, A], dt.bfloat16)   # (kch, a)
            dma(out=wdecT_sb[:],
                in_=wdecT[:].rearrange("(kc kp) a -> kp kc a", kp=128))
            wrow_sb = const.tile([128, 4], dt.bfloat16)       # Wfull (signed)
            dma(out=wrow_sb[:], in_=wfullb[:].rearrange("(c p) -> p c", p=128))
            bea_row = const.tile([1, A], dt.bfloat16)
            dma(out=bea_row[:], in_=beab[:])
            wdiff_sb = const.tile([128, 12], dt.bfloat16)
            dma(out=wdiff_sb[:], in_=wdiffT[:].rearrange("(c p) -> p c", p=128))
            bdiff_sb = const.tile([1, 1], dt.bfloat16)
            dma(out=bdiff_sb[:], in_=bdiffb[:])
            bfcrow_sb = const.tile([1, VCH * 128], dt.bfloat16)
            dma(out=bfcrow_sb[:], in_=bfcp[:])
            ones_sb = const.tile([128, 1], dt.bfloat16)
            nc.vector.memset(ones_sb[:], 1.0)

            # ---------- stage 0: feats = sum_h enc (1/16 folded in Wih1) ----
            with nc.allow_low_precision(reason="bf16 feats sum of 16 values"):
                for b_ in range(NB):
                    rsrc = enc_ep_sb[:, b_, :, :].rearrange(
                        "p ec (h w) -> p ec w h", h=HH)
                    nc.vector.tensor_reduce(
                        out=feats[:, :, b_, :], in_=rsrc,
                        axis=mybir.AxisListType.X, op=ALU.add)

            # ---------- layer-1 input projections (all t, N=128) ----------
            for d_ in (0, 1):
                for mp in range(8):
                    pt = ps_mm.tile([128, 512], dt.float32, tag="pmm")
                    for half in (0, 1):
                        mch = 2 * mp + half
                        sl = pt[:, half * 128:(half + 1) * 128]
                        for kc in range(4):
                            nc.tensor.matmul(
                                sl,
                                wih1_sb[d_][0][:, kc,
                                               mch * 128:(mch + 1) * 128],
                                feats[:, kc, :, :], start=(kc == 0),
                                stop=False)
                        nc.tensor.matmul(
                            sl, b1row[d_][0:1, mch * 128:(mch + 1) * 128],
                            ones1b_sb[:], start=False, stop=True)
                    nc.vector.tensor_copy(
                        Xp1[d_][:, 2 * mp:2 * mp + 2, :, :]
                        .rearrange("p m b w -> p (m b w)"), pt[:, 0:256])

            # ---------- LSTM fused step pair ----------
            # Gate blocks host-permuted to (i, f, o, g):
            # ch 0-3=i, 4-7=f, 8-11=o, 12-15=g.
            # psum/pre/ga layout: [128, cell(2), ch(16), b(8)]; both cells'
            # elementwise fused into single ops (DVE/ACT ops are the scarce
            # resource on this platform).
            def step_pair(wsb, xps, Hs, c_tile, s, lgi):
                # separate psum tiles per gate group so the activations can
                # start as soon as THEIR accumulation stops (deps are
                # tile-granular): g first (tanh feeds ig), then i,f, then o
                # (sigmoid(o) hides behind the c update)
                pg_g = ps_g.tile([128, 2, 4, NB], dt.float32, tag="pgg",
                                 bufs=1, name="pgg")
                pg_if = ps_g.tile([128, 2, 8, NB], dt.float32, tag="pgif",
                                  bufs=1, name="pgif")
                pg_o = ps_g.tile([128, 2, 4, NB], dt.float32, tag="pgo",
                                 bufs=1, name="pgo")
                for pt_, mlo, nch in ((pg_g, 12, 4), (pg_if, 0, 8),
                                      (pg_o, 8, 4)):
                    for d_ in (0, 1):
                        t_log = s if d_ == 0 else T - 1 - s
                        t_prev = t_log - 1 if d_ == 0 else t_log + 1
                        h_prev = None if s == 0 else Hs[d_][:, :, t_prev, :]
                        for j in range(nch):
                            mch = mlo + j
                            if h_prev is not None:
                                for kc in range(4):
                                    nc.tensor.matmul(
                                        pt_[:, d_, j, :],
                                        wsb[d_][:, kc,
                                                mch * 128:(mch + 1) * 128],
                                        h_prev[:, kc, :],
                                        start=(kc == 0), stop=False)
                            # += Xp via identity matmul (PE op, not DVE add)
                            nc.tensor.matmul(
                                pt_[:, d_, j, :], eye_sb[:],
                                xps[d_][:, mch, :],
                                start=(s == 0), stop=True)
                ga_g = work.tile([128, 2, 4, NB], dt.float32, tag="gag",
                                 name="gag")
                ga_if = work.tile([128, 2, 8, NB], dt.float32, tag="gaif",
                                  bufs=3, name="gaif")
                ga_o = work.tile([128, 2, 4, NB], dt.float32, tag="gao",
                                 name="gao")
                nc.scalar.activation(ga_g[:], pg_g[:], AF.Tanh)
                nc.scalar.activation(ga_if[:], pg_if[:], AF.Sigmoid)
                nc.scalar.activation(ga_o[:], pg_o[:], AF.Sigmoid)
                ig = work.tile([128, 2, 4, NB], dt.float32, tag="ig",
                               name="ig")
                nc.vector.tensor_tensor(out=ig[:], in0=ga_if[:, :, 0:4, :],
                                        in1=ga_g[:], op=ALU.mult)
                if s == 0:
                    nc.vector.tensor_copy(c_tile[:], ig[:])
                else:
                    nc.vector.tensor_tensor(out=c_tile[:], in0=c_tile[:],
                                            in1=ga_if[:, :, 4:8, :],
                                            op=ALU.mult)
                    nc.vector.tensor_tensor(out=c_tile[:], in0=c_tile[:],
                                            in1=ig[:], op=ALU.add)
                th = work.tile([128, 2, 4, NB], dt.float32, tag="th",
                               name="th")
                nc.scalar.activation(th[:], c_tile[:], AF.Tanh)
                for d_ in (0, 1):
                    t_log = s if d_ == 0 else T - 1 - s
                    eng = nc.vector if d_ == 0 else nc.gpsimd
                    eng.tensor_tensor(out=Hs[d_][:, :, t_log, :],
                                      in0=th[:, d_, :, :],
                                      in1=ga_o[:, d_, :, :],
                                      op=ALU.mult)

            # ---------- layer-1 recurrence ----------
            c1 = work.tile([128, 2, 4, NB], dt.float32, tag="c1", bufs=1,
                           name="c1")
            for s in range(T):
                step_pair(whh1_view, {
                    0: Xp1[0][:, :, :, s],
                    1: Xp1[1][:, :, :, T - 1 - s]}, H1, c1, s, 1)

            # ---------- att1w = Wenc^T enc  (in the post-L1 load hole) -----
            for ac in range(4):
                for bblk in range(4):
                    pt = ps_mm.tile([128, 512], dt.float32, tag="pmm",
                                    name="pta1")
                    for ec in range(4):
                        nc.tensor.matmul(
                            pt[:],
                            wencT_sb[:, ec, ac * 128:(ac + 1) * 128],
                            enc_ep_sb[:, 2 * bblk:2 * bblk + 2, ec, :],
                            start=(ec == 0), stop=(ec == 3))
                    if (ac + bblk) % 2 == 0:
                        nc.vector.tensor_copy(
                            att1w[:, 2 * bblk:2 * bblk + 2, ac, :], pt[:])
                    else:
                        nc.scalar.copy(
                            att1w[:, 2 * bblk:2 * bblk + 2, ac, :], pt[:])


            # ---------- layer-2 input projections ----------
            wih2_sb = {d_: load_w(wih2[d_], 8) for d_ in (0, 1)}
            Xp2 = {d_: const.tile([128, 16, T, NB], dt.bfloat16,
                                  tag=f"xp_{d_}", bufs=1, name=f"Xp2_{d_}")
                   for d_ in (0, 1)}                          # (gch, t, b)
            for d_ in (0, 1):
                for mp in range(8):
                    pt = ps_mm.tile([128, 512], dt.float32, tag="pmm")
                    for half in (0, 1):
                        mch = 2 * mp + half
                        sl = pt[:, half * 128:(half + 1) * 128]
                        for kc in range(8):
                            rhs = (H1[0] if kc < 4 else H1[1])[:, kc % 4, :, :]
                            nc.tensor.matmul(
                                sl,
                                wih2_sb[d_][kc // 4][:, kc % 4,
                                                     mch * 128:(mch + 1) * 128],
                                rhs, start=(kc == 0), stop=False)
                        nc.tensor.matmul(
                            sl, b2row[d_][0:1, mch * 128:(mch + 1) * 128],
                            ones1b_sb[:], start=False, stop=True)
                    nc.vector.tensor_copy(
                        Xp2[d_][:, 2 * mp:2 * mp + 2, :, :]
                        .rearrange("p m t b -> p (m t b)"), pt[:, 0:256])

            whh2_sb = {d_: load_w(whh2[d_], 4) for d_ in (0, 1)}
            whh2_view = {d_: whh2_sb[d_][0] for d_ in (0, 1)}

            # ---------- attention helpers (used interleaved + deferred) ----
            sc_t = [ps_sc.tile([128, 128], dt.float32, tag=f"sc{ph}",
                               name=f"scps{ph}") for ph in range(2)]

            def h2rhs(kc):
                return (H2[0] if kc < 4 else H2[1])[:, kc % 4, :, :] \
                    .rearrange("p t b -> p b t")

            def emit_att2_col(tt):
                # att2pb[:, :, (b,tt)] = Wdec^T h2[tt] + bea, via psum
                pa2 = ps_g.tile([128, 4, NB], dt.float32, tag="pgo",
                                bufs=1, name="pa2")
                for ac in range(4):
                    for kc in range(8):
                        nc.tensor.matmul(
                            pa2[:, ac, :],
                            wdecT_sb[:, kc, ac * 128:(ac + 1) * 128],
                            (H2[0] if kc < 4 else H2[1])[:, kc % 4, tt, :],
                            start=(kc == 0), stop=False)
                    nc.tensor.matmul(
                        pa2[:, ac, :], bea_row[0:1, ac * 128:(ac + 1) * 128],
                        ones1b_sb[0:1, 0:NB], start=False, stop=True)
                nc.vector.tensor_copy(att2pb_r[:, :, tt, :], pa2[:])

            def emit_rw_col(b_, tt, eng):
                col = b_ * T + tt
                for ac in range(4):
                    rw = rwp.tile([128, PP], dt.bfloat16, tag="rw")
                    if eng == 0:
                        nc.vector.tensor_scalar(
                            out=rw[:], in0=att1w[:, b_, ac, :],
                            scalar1=att2pb[:, ac, col:col + 1],
                            scalar2=0.0, op0=ALU.add, op1=ALU.max)
                    elif eng == 1:
                        nc.scalar.activation(
                            rw[:], att1w[:, b_, ac, :], AF.Relu,
                            bias=att2pb[:, ac, col:col + 1])
                    else:
                        nc.gpsimd.tensor_scalar(
                            out=rw[:], in0=att1w[:, b_, ac, :],
                            scalar1=att2pb[:, ac, col:col + 1],
                            scalar2=0.0, op0=ALU.add, op1=ALU.max)
                    for ph in range(2):
                        nc.tensor.matmul(
                            sc_t[ph][:, col:col + 1],
                            rw[:, ph * 128:(ph + 1) * 128],
                            wrow_sb[:, ac:ac + 1],
                            start=(ac == 0), stop=(ac == 3))

            done_cols = set()

            # ---------- layer-2 recurrence (att2/relu interleaved) ----------
            c2 = work.tile([128, 2, 4, NB], dt.float32, tag="c2", bufs=1,
                           name="c2")
            for s in range(T):
                step_pair(whh2_view, {
                    0: Xp2[0][:, :, s, :],
                    1: Xp2[1][:, :, T - 1 - s, :]}, H2, c2, s, 2)
                if s >= 8:
                    # att2 for the two completed timesteps: PE matmuls (~free
                    # inside the latency-bound loop) + one small DVE copy each
                    for tt in (s, T - 1 - s):
                        emit_att2_col(tt)
                    # interleave a few relu columns (DVE-heavy; keep ACT
                    # free for the recurrence sigmoids/tanhs)
                    ivl = [(0, s, 0), (1, s, 0), (2, s, 0),
                           (3, s, 0), (4, s, 2)]
                    if s % 2 == 1:
                        ivl.append((5, s, 1))
                    for b_, tt, eng in ivl:
                        emit_rw_col(b_, tt, eng)
                        done_cols.add((b_, tt))

            # switch the ACT table to exp_and_others here (ACT is ~half
            # idle); softmax exp + gate exp then need no load. relu/tanh/
            # copy/identity are in both tables.
            dmy = work.tile([1, 1], dt.float32, tag="dmy", bufs=1, name="dmy")
            nc.scalar.activation(dmy[:], ones1_sb[0:1, 0:1], AF.Exp)

            # ---------- Hw = hidden in (b,t)-column layout (for lhsT) ------
            for kc in range(8):
                src = h2rhs(kc)
                if kc < 4:
                    nc.vector.tensor_copy(Hw[:, kc, :], src)
                elif kc < 6:
                    nc.scalar.copy(Hw[:, kc, :], src)
                else:
                    nc.gpsimd.tensor_copy(Hw[:, kc, :], src)

            # ---------- fc hidden-part: U[vb] = hid^T Wfc_h + bfc ----------
            for vb in range(NVB):
                v0 = vb * 512
                vn = min(512, V - v0)
                wt = wfcp.tile([128, 8, 512], dt.bfloat16, tag="wfcU",
                               name="wtU")
                dma(out=wt[:, :, 0:vn],
                    in_=wfcT[0:2 * D, v0:v0 + vn]
                    .rearrange("(kc kp) v -> kp kc v", kp=128))
                pt = ps_mm.tile([128, 512], dt.float32, tag="pmm")
                for kc in range(8):
                    nc.tensor.matmul(pt[:, 0:vn], Hw[:, kc, :],
                                     wt[:, kc, 0:vn],
                                     start=(kc == 0), stop=False)
                nc.tensor.matmul(pt[:, 0:vn], ones1b_sb[:],
                                 bfcrow_sb[0:1, v0:v0 + vn],
                                 start=False, stop=True)
                nc.scalar.copy(U_sb[vb][:, 0:vn], pt[:, 0:vn])

            # prefetch the first awe-part weight tiles on the ACT dma queue
            # (emitted before the relu block so the ACT sequencer issues them
            # before it starts chewing on relu ops; they stream during it)
            wtA = []

            def load_A(vb):
                wt = wfcp.tile([128, 4, 512], dt.bfloat16, tag="wfcA",
                               name="wtA")
                v0 = vb * 512
                vn = min(512, V - v0)
                nc.scalar.dma_start(
                    out=wt[:, :, 0:vn],
                    in_=wfcT[2 * D:F, v0:v0 + vn]
                    .rearrange("(kc kp) v -> kp kc v", kp=128))
                wtA.append(wt)

            load_A(0)
            load_A(1)

            # ---------- deferred relu columns ----------
            # engine shares ~ DVE 0.68 / ACT 0.18 / Pool 0.14 by op cost
            rem = [(b_, tt) for b_ in range(NB) for tt in range(T)
                   if (b_, tt) not in done_cols]
            accD = accA = accP = 0.0
            for b_, tt in rem:
                # weighted round-robin: pick engine with least accumulated ns
                costs = (accD + 4 * 127.0, accA + 4 * 398.0, accP + 4 * 544.0)
                eng = int(np.argmin(costs))
                if eng == 0:
                    accD = costs[0]
                elif eng == 1:
                    accA = costs[1]
                else:
                    accP = costs[2]
                emit_rw_col(b_, tt, eng)

            # ---------- softmax over p (stay transposed) ----------
            for ph in range(2):
                nc.scalar.activation(E_sb[:, ph, :], sc_t[ph][:], AF.Exp)
            sums = ps_sc.tile([1, 128], dt.float32, tag="sc0")
            for ph in range(2):
                nc.tensor.matmul(sums[:], ones_sb[:], E_sb[:, ph, :],
                                 start=(ph == 0), stop=(ph == 1))
            nc.vector.reciprocal(recip_sb[:], sums[:])
            recip_bc = ps_g.tile([128, 128], dt.float32, tag="pgo",
                                 bufs=1, name="recip_bc")
            nc.tensor.matmul(recip_bc[:], ones1_sb[:], recip_sb[:],
                             start=True, stop=True)
            for ph in range(2):
                nc.vector.tensor_tensor(out=alphaT[:, ph, :],
                                        in0=E_sb[:, ph, :],
                                        in1=recip_bc[:], op=ALU.mult)

            # ---------- awe^T[e,(b,t)] ----------
            for ec in range(4):
                pa = ps_g.tile([128, 128], dt.float32, tag="pgo", bufs=1)
                for b_ in range(NB):
                    for pc in range(2):
                        nc.tensor.matmul(
                            pa[:, b_ * T:(b_ + 1) * T],
                            enc_pe_sb[:, b_, pc, ec * 128:(ec + 1) * 128],
                            alphaT[:, pc, b_ * T:(b_ + 1) * T],
                            start=(pc == 0), stop=(pc == 1))
                nc.vector.tensor_copy(aweT[:, ec, :], pa[:])

            # ---------- gate: g01T[bt, 0:2] = sigmoid(+-(wdiff.feat+bd)) ---
            def fc_feat_rhs(kc):
                return Hw[:, kc, :] if kc < 8 else aweT[:, kc - 8, :]

            glT = ps_g.tile([128, 1], dt.float32, tag="pgo", bufs=1, name="glt")
            for kc in range(12):
                nc.tensor.matmul(glT[:], fc_feat_rhs(kc),
                                 wdiff_sb[:, kc:kc + 1],
                                 start=(kc == 0), stop=False)
            nc.tensor.matmul(glT[:], ones1b_sb[:], bdiff_sb[:],
                             start=False, stop=True)
            gex = work.tile([128, 1], dt.float32, tag="gex", bufs=1,
                            name="gex")
            nc.scalar.activation(gex[:], glT[:], AF.Exp, scale=-1.0)
            nc.vector.tensor_scalar(out=gex[:], in0=gex[:], scalar1=1.0,
                                    scalar2=None, op0=ALU.add)
            nc.vector.reciprocal(g01T[:, 0:1], gex[:])
            nc.vector.tensor_scalar(out=g01T[:, 1:2], in0=g01T[:, 0:1],
                                    scalar1=-1.0, scalar2=1.0,
                                    op0=ALU.mult, op1=ALU.add)

            # ---------- fc awe-part + gate combine + writeback ----------
            for vb in range(NVB):
                v0 = vb * 512
                vn = min(512, V - v0)
                if vb + 2 < NVB:
                    load_A(vb + 2)
                wt = wtA[vb]
                pt = ps_mm.tile([128, 512], dt.float32, tag="pmm")
                for kc in range(4):
                    nc.tensor.matmul(pt[:, 0:vn], aweT[:, kc, :],
                                     wt[:, kc, 0:vn],
                                     start=(kc == 0), stop=False)
                nc.tensor.matmul(pt[:, 0:vn], ones1b_sb[:],
                                 bfcrow_sb[0:1, v0:v0 + vn],
                                 start=False, stop=True)
                ost = outp.tile([128, 512], dt.float32, tag="ost")
                nc.scalar.activation(ost[:, 0:vn], U_sb[vb][:, 0:vn],
                                     AF.Identity, scale=g01T[:, 0:1])
                nc.vector.scalar_tensor_tensor(
                    out=ost[:, 0:vn], in0=pt[:, 0:vn],
                    scalar=g01T[:, 1:2], in1=ost[:, 0:vn],
                    op0=ALU.mult, op1=ALU.add)
                dst = bass.AP(tensor=out_t[:].tensor, offset=v0,
                              ap=[[V, 128], [1, vn]])
                dma(out=dst, in_=ost[:, 0:vn])

    nc.compile()
    return nc


def _host_prep(inputs):
    f32 = np.float32

    def bf(x):
        return np.ascontiguousarray(np.asarray(x, f32).astype(BF))

    enc = np.asarray(inputs["encoder_out"], f32)
    enc_p = enc.reshape(B, E, PP)

    # permute gate blocks (i,f,g,o) -> (i,f,o,g) so one sigmoid spans i,f,o
    gp = np.r_[0:2 * D, 3 * D:4 * D, 2 * D:3 * D]

    common = {}
    common["wih1f"] = bf(np.asarray(inputs["Wih1"], f32).T[:, gp] / HH)
    common["wih1r"] = bf(np.asarray(inputs["Wih1r"], f32).T[:, gp] / HH)
    common["whh1f"] = bf(np.asarray(inputs["Whh1"], f32).T[:, gp])
    common["whh1r"] = bf(np.asarray(inputs["Whh1r"], f32).T[:, gp])
    common["wih2f"] = bf(np.asarray(inputs["Wih2"], f32).T[:, gp])
    common["wih2r"] = bf(np.asarray(inputs["Wih2r"], f32).T[:, gp])
    common["whh2f"] = bf(np.asarray(inputs["Whh2"], f32).T[:, gp])
    common["whh2r"] = bf(np.asarray(inputs["Whh2r"], f32).T[:, gp])
    common["b1f"] = bf(np.asarray(inputs["bih1"] + inputs["bhh1"], f32)[gp])
    common["b1r"] = bf(np.asarray(inputs["bih1r"] + inputs["bhh1r"], f32)[gp])
    common["b2f"] = bf(np.asarray(inputs["bih2"] + inputs["bhh2"], f32)[gp])
    common["b2r"] = bf(np.asarray(inputs["bih2r"] + inputs["bhh2r"], f32)[gp])
    common["wencT"] = bf(np.asarray(inputs["Wenc"], f32).T)
    common["wdecT"] = bf(np.asarray(inputs["Wdec"], f32).T)
    common["beab"] = bf(np.asarray(inputs["benc"] + inputs["bdec"], f32))
    common["wfullb"] = bf(np.asarray(inputs["Wfull"], f32)[0])
    wg = np.asarray(inputs["Wg"], f32)
    common["wdiffT"] = bf(wg[0] - wg[1])
    bd = float(np.asarray(inputs["bg"], f32)[0] - np.asarray(inputs["bg"], f32)[1])
    common["bdiffb"] = bf(np.array([[bd]], f32))
    common["eye128"] = bf(np.eye(128, dtype=f32))
    common["wfcT"] = bf(np.asarray(inputs["Wfc"], f32).T)
    bfc = np.zeros(VCH * 128, f32)
    bfc[:V] = np.asarray(inputs["bfc"], f32)
    common["bfcp"] = bf(bfc)

    in_maps = []
    for c in range(NCORES):
        m = dict(common)
        sl = enc_p[c * NB:(c + 1) * NB]
        m["enc_ep"] = bf(sl)
        m["enc_pe"] = bf(np.ascontiguousarray(sl.transpose(0, 2, 1)))
        in_maps.append(m)
    return in_maps


def _get_program():
    if "nc" not in _prog_cache:
        _prog_cache["nc"] = _build_program()
    return _prog_cache["nc"]


def kernel(**inputs):
    from concourse.bass_utils import run_bass_kernel_spmd

    nc = _get_program()
    in_maps = _host_prep(inputs)
    res = run_bass_kernel_spmd(nc, in_maps, list(range(NCORES)))
    # per-core result is [b, t, v]; assemble to (T, B, V)
    out = np.concatenate(
        [res.results[c]["out"].transpose(1, 0, 2) for c in range(NCORES)],
        axis=1)
    return np.ascontiguousarray(out, np.float32)



# revision 25
# speedup vs baseline: 1.1464x; 1.1464x over previous
"""Trainium2 Bass kernel for DecoderWithAttention (bidirectional 2-layer LSTM +
additive attention + gated fc), data-parallel over batch across 8 NeuronCores.

Shapes (hardcoded): encoder_out (64, 512, 16, 16), T=16, D=A=512, V=5000.
Per core: 8 batches, full network, weights replicated (no collectives under
this axon terminal, so each core is fully independent).

Major layout/optimization decisions (v2):
  - All large weights (Wih*/Whh*/Wenc/Wdec/Wfc) are stored+streamed as
    float8 e3m4, host-scaled by a fixed power-of-2 (SCL=32, clipped) so the
    fp8 mantissa window is centered; dequant is folded into ACT scale params
    (gates: sigmoid/tanh(psum/SCL); softmax: exp(sc/SCL); fc: 1/SCL folded
    into the gate coefficients g0/g1).  Halves the ~43MB/core DMA to ~20MB.
  - feats (AdaptiveAvgPool over H) is computed host-side (input prep),
    removing the on-chip reduce and letting L1 start ~10us earlier.
  - LSTM step: psum groups (g | ifo) -> 3 ACT ops/step (tanh_g, sig_ifo,
    tanh_c); both directions fused in each op; h written by DVE (fwd) and
    Pool (rev).  Xp psum->SBUF copies ride on Pool/DVE.
  - Weights pinned in SBUF (no rotation pool), so every weight DMA issues
    back-to-back with no WAR stalls; wfc hidden/awe parts stream through
    small fp8 rotation pools ordered so the sync queue never head-blocks.
  - Attention: att1w=Wenc^T enc precomputed (scheduler lifts it into L1-rec
    PE idle); att2 per-timestep inside the L2 loop; relu/score columns for
    b0/b1 interleaved into the L2 loop (DVE/ACT/Pool), the rest deferred and
    engine-balanced; softmax+awe pipelined per-batch behind the relu columns.
  - fc factorized around the gate (g0*(hid Wh+bfc) + g1*(awe Wa+bfc)); the
    hidden part streams right after the L2 recurrence; gate sigmoid folds
    1/SCL; output written bf16 (host upcasts to f32).
"""

import numpy as np
import ml_dtypes

BF = ml_dtypes.bfloat16
E3 = ml_dtypes.float8_e3m4
B, E, HH, WW = 64, 512, 16, 16
T = WW          # 16 timesteps
PP = HH * WW    # 256 attention positions
D = 512
A = 512
V = 5000
G = 4 * D
NB = 8          # batches per core
NCORES = 8
F = 2 * D + E   # 1536
VCH = (V + 127) // 128  # 40
NVB = 10        # fc v-blocks of 512

SCL = 32.0      # fixed power-of-2 fp8 scale for all quantized weights
DQ = 1.0 / SCL  # dequant factor (exact)
E3MAX = 15.5

_prog_cache = {}


def _build_program():
    import concourse.bass as bass
    import concourse.bacc as bacc
    import concourse.mybir as mybir
    import concourse.tile as tile

    dt = mybir.dt
    AF = mybir.ActivationFunctionType
    ALU = mybir.AluOpType

    nc = bacc.Bacc("TRN2", target_bir_lowering=False, debug=False,
                   num_devices=NCORES, dynamic_dma_scratch_size=2048)

    def din(name, shape, d=dt.bfloat16):
        return nc.dram_tensor(name, shape, d, kind="ExternalInput")

    f8 = dt.float8e3
    featsb = din("featsb", [E, NB, T])            # host mean over H, bf16
    enc_ep = din("enc_ep", [NB, E, PP])           # [b, e, p]
    enc_pe = din("enc_pe", [NB, PP, E])           # [b, p, e]
    wih1 = {0: din("wih1f", [E, G], f8), 1: din("wih1r", [E, G], f8)}
    whh1 = {0: din("whh1f", [D, G], f8), 1: din("whh1r", [D, G], f8)}
    wih2 = {0: din("wih2f", [2 * D, G], f8), 1: din("wih2r", [2 * D, G], f8)}
    whh2 = {0: din("whh2f", [D, G], f8), 1: din("whh2r", [D, G], f8)}
    b1 = {0: din("b1f", [G]), 1: din("b1r", [G])}   # (bih+bhh)*SCL bf16
    b2 = {0: din("b2f", [G]), 1: din("b2r", [G])}
    wencT = din("wencT", [E, A], f8)
    wdecT = din("wdecT", [2 * D, A], f8)
    beab = din("beab", [A])                       # (benc+bdec)*SCL, bf16
    wfullb = din("wfullb", [A])                   # Wfull[0] bf16 (signed)
    wdiffT = din("wdiffT", [F])                   # Wg[0]-Wg[1] bf16
    bdiffb = din("bdiffb", [1, 1])                # bg0-bg1 bf16
    eye128 = din("eye128", [128, 128])
    wfcT = din("wfcT", [F, V], f8)
    bfcp = din("bfcp", [VCH * 128])               # bfc*SCL, bf16 padded
    out_t = nc.dram_tensor("out", [NB, T, V], dt.bfloat16,
                           kind="ExternalOutput")

    with tile.TileContext(nc) as tc:
        with (
            tc.tile_pool(name="const", bufs=1) as const,
            tc.tile_pool(name="work", bufs=4) as work,
            tc.tile_pool(name="rwp", bufs=10) as rwp,
            tc.tile_pool(name="wfcp", bufs=2) as wfcp,
            tc.tile_pool(name="outp", bufs=3) as outp,
            tc.tile_pool(name="ps_g", bufs=1, space="PSUM") as ps_g,
            tc.tile_pool(name="ps_mm", bufs=3, space="PSUM") as ps_mm,
            tc.tile_pool(name="ps_sc", bufs=1, space="PSUM") as ps_sc,
        ):
            dma = nc.sync.dma_start

            # ---------------- DMA section (sync-queue program order) -------
            # tiny tiles the early phases depend on go first
            feats = const.tile([128, 4, NB, T], dt.bfloat16)   # (ech, b, t)
            dma(out=feats[:],
                in_=featsb[:].rearrange("(ec ep) b t -> ep ec b t", ep=128))
            eye_sb = const.tile([128, 128], dt.bfloat16)
            dma(out=eye_sb[:], in_=eye128[:])
            b1row, b2row = {}, {}
            for d_ in (0, 1):
                b1row[d_] = const.tile([1, G], dt.bfloat16, tag=f"b1r_{d_}",
                                       bufs=1, name=f"b1row{d_}")
                dma(out=b1row[d_][:], in_=b1[d_][:])

            # pinned fp8 LSTM weights.  L1 weights are split into two
            # half-G tiles each so the first projection / recurrence matmuls
            # only wait on the first half-MB DMA.
            def wload(dram, kchunks, nm, splits=1):
                gs = G // splits
                ts = []
                for i in range(splits):
                    t_ = const.tile([128, kchunks, gs], f8, name=f"{nm}_{i}")
                    dma(out=t_[:],
                        in_=dram[:, i * gs:(i + 1) * gs]
                        .rearrange("(kc kp) g -> kp kc g", kp=128))
                    ts.append(t_)
                return ts

            def wsl(wsb, d_, kc, mch):
                # weight slice [128, 128] for gate chunk mch
                ts = wsb[d_]
                n = len(ts)
                per = 16 // n
                t_ = ts[mch // per]
                j = mch % per
                return t_[:, kc, j * 128:(j + 1) * 128]

            wih1_sb = {d_: wload(wih1[d_], 4, f"wih1_{d_}", 2)
                       for d_ in (0, 1)}
            whh1_sb = {d_: wload(whh1[d_], 4, f"whh1_{d_}", 2)
                       for d_ in (0, 1)}
            wencT_sb = const.tile([128, 4, A], f8)            # (ech, a)
            dma(out=wencT_sb[:],
                in_=wencT[:].rearrange("(ec ep) a -> ep ec a", ep=128))
            enc_ep_sb = const.tile([128, NB, 4, PP], dt.bfloat16,
                                   tag="encbuf", bufs=1, name="enc_ep")
            for bh in (0, 1):
                dma(out=enc_ep_sb[:, 4 * bh:4 * bh + 4, :, :],
                    in_=enc_ep[4 * bh:4 * bh + 4]
                    .rearrange("b (ec ep) p -> ep b ec p", ep=128))
            for d_ in (0, 1):
                b2row[d_] = const.tile([1, G], dt.bfloat16, tag=f"b2r_{d_}",
                                       name=f"b2row{d_}")
                dma(out=b2row[d_][:], in_=b2[d_][:])
            wih2_sb = {d_: wload(wih2[d_], 8, f"wih2_{d_}") for d_ in (0, 1)}
            whh2_sb = {d_: wload(whh2[d_], 4, f"whh2_{d_}") for d_ in (0, 1)}
            # prime the tanh/sigmoid ACT table during the DMA head so the
            # first gate activation doesn't eat a 1.3us table load
            prime = work.tile([1, 1], dt.float32, tag="dmy", bufs=1,
                              name="prime")
            nc.scalar.activation(prime[:], eye_sb[0:1, 0:1], AF.Tanh)
            wdecT_sb = const.tile([128, 8, A], f8)            # (kch, a)
            dma(out=wdecT_sb[:],
                in_=wdecT[:].rearrange("(kc kp) a -> kp kc a", kp=128))
            wrow_sb = const.tile([128, 4], dt.bfloat16)       # Wfull (signed)
            dma(out=wrow_sb[:], in_=wfullb[:].rearrange("(c p) -> p c", p=128))
            bea_row = const.tile([1, A], dt.bfloat16)
            dma(out=bea_row[:], in_=beab[:])
            wdiff_sb = const.tile([128, 12], dt.bfloat16)
            dma(out=wdiff_sb[:], in_=wdiffT[:].rearrange("(c p) -> p c", p=128))
            bdiff_sb = const.tile([1, 1], dt.bfloat16)
            dma(out=bdiff_sb[:], in_=bdiffb[:])
            bfcrow_sb = const.tile([1, VCH * 128], dt.bfloat16)
            dma(out=bfcrow_sb[:], in_=bfcp[:])
            # enc_pe shares the enc_ep slot (same 16KB); its DMA waits until
            # the att1w matmuls (the only enc_ep readers) are done
            enc_pe_sb = const.tile([128, NB, 2, E], dt.bfloat16,
                                   tag="encbuf", name="enc_pe")
            dma(out=enc_pe_sb[:],
                in_=enc_pe[:].rearrange("b (pc pp) e -> pp b pc e", pp=128))

            # fc weight streams (fp8). wfcU rotation: first 6 issue during
            # the recurrences, the rest pace with the U matmuls.  wfcA's
            # first 4 issue early; the rest pace with the A matmuls.  Order
            # keeps sync-queue head-blocking monotone with need times.
            wtU, wtA = [], []

            def load_U(vb):
                wt = wfcp.tile([128, 8, 512], f8, tag="wfcU", bufs=5,
                               name="wtU")
                v0 = vb * 512
                vn = min(512, V - v0)
                dma(out=wt[:, :, 0:vn],
                    in_=wfcT[0:2 * D, v0:v0 + vn]
                    .rearrange("(kc kp) v -> kp kc v", kp=128))
                wtU.append(wt)

            def load_A(vb):
                wt = wfcp.tile([128, 4, 512], f8, tag="wfcA", bufs=2,
                               name="wtA")
                v0 = vb * 512
                vn = min(512, V - v0)
                dma(out=wt[:, :, 0:vn],
                    in_=wfcT[2 * D:F, v0:v0 + vn]
                    .rearrange("(kc kp) v -> kp kc v", kp=128))
                wtA.append(wt)

            for vb in range(5):
                load_U(vb)
            for vb in range(2):
                load_A(vb)
            for vb in range(5, NVB):
                load_U(vb)
            for vb in range(2, NVB):
                load_A(vb)

            # ---------------- persistent SBUF state ----------------
            Xp1 = {d_: const.tile([128, 16, NB, T], dt.bfloat16,
                                  tag=f"xp1_{d_}", bufs=1, name=f"Xp1_{d_}")
                   for d_ in (0, 1)}                          # (gch, b, t)
            Xp2 = {d_: const.tile([128, 16, T, NB], dt.bfloat16,
                                  tag=f"xp2_{d_}", bufs=1, name=f"Xp2_{d_}")
                   for d_ in (0, 1)}                          # (gch, t, b)
            H1 = {d_: const.tile([128, 4, T, NB], dt.bfloat16,
                                 tag=f"h1_{d_}", name=f"H1_{d_}")
                  for d_ in (0, 1)}                           # (dch, t, b)
            H2 = {d_: const.tile([128, 4, T, NB], dt.bfloat16,
                                 tag=f"h2_{d_}", name=f"H2_{d_}")
                  for d_ in (0, 1)}
            att1w = const.tile([128, NB, 4, PP], dt.bfloat16)  # (b, ach, p)
            att2pb = const.tile([128, 4, 128], dt.float32)     # (ach, (b,t))
            att2pb_r = att2pb[:].rearrange("p a (b t) -> p a t b", t=T)
            E_sb = const.tile([128, 2, 128], dt.bfloat16)      # exp(sc/SCL)
            alphaT = const.tile([128, 2, 128], dt.bfloat16)    # (pch, (b,t))
            aweT = const.tile([128, 4, 128], dt.bfloat16)      # (ech, (b,t))
            recip_sb = const.tile([1, 128], dt.float32)
            U_sb = [const.tile([128, 512], dt.bfloat16, tag=f"u{vb}",
                               name=f"U{vb}") for vb in range(NVB)]
            g01T = const.tile([128, 2], dt.float32)
            ones1_sb = const.tile([1, 128], dt.float32)
            nc.vector.memset(ones1_sb[:], 1.0)
            ones1b_sb = const.tile([1, 128], dt.bfloat16)
            nc.vector.memset(ones1b_sb[:], 1.0)
            ones_sb = const.tile([128, 1], dt.bfloat16)
            nc.vector.memset(ones_sb[:], 1.0)

            Hw = const.tile([128, 8, 128], dt.bfloat16)  # (kch, (b,t))

            def hid_cols(kc):
                return Hw[:, kc, :]

            # ---------- layer-1 input projections (all t, N=128) ----------
            for d_ in (0, 1):
                for mp in range(8):
                    pt = ps_mm.tile([128, 512], dt.float32, tag="pmm")
                    for half in (0, 1):
                        mch = 2 * mp + half
                        sl = pt[:, half * 128:(half + 1) * 128]
                        for kc in range(4):
                            nc.tensor.matmul(
                                sl,
                                wsl(wih1_sb, d_, kc, mch),
                                feats[:, kc, :, :], start=(kc == 0),
                                stop=False)
                        nc.tensor.matmul(
                            sl, b1row[d_][0:1, mch * 128:(mch + 1) * 128],
                            ones1b_sb[:], start=False, stop=True)
                    # gpsimd cannot read PSUM on hw; alternate DVE/ACT
                    if mp % 2 == 0:
                        nc.vector.tensor_copy(
                            Xp1[d_][:, 2 * mp:2 * mp + 2, :, :]
                            .rearrange("p m b w -> p (m b w)"), pt[:, 0:256])
                    else:
                        nc.scalar.copy(
                            Xp1[d_][:, 2 * mp:2 * mp + 2, :, :]
                            .rearrange("p m b w -> p (m b w)"), pt[:, 0:256])

            # ---------- LSTM fused step pair ----------
            # Gate blocks host-permuted to (i, f, o, g):
            # ch 0-3=i, 4-7=f, 8-11=o, 12-15=g.
            # psum groups: g (tanh feeds ig first) | ifo (single sigmoid).
            def step_pair(wsb, xps, Hs, c_tile, s):
                pg_g = ps_g.tile([128, 2, 4, NB], dt.float32, tag="pgg",
                                 bufs=1, name="pgg")
                pg_ifo = ps_g.tile([128, 2, 12, NB], dt.float32, tag="pgifo",
                                   bufs=1, name="pgifo")
                for pt_, mlo, nch in ((pg_g, 12, 4), (pg_ifo, 0, 12)):
                    for d_ in (0, 1):
                        t_log = s if d_ == 0 else T - 1 - s
                        t_prev = t_log - 1 if d_ == 0 else t_log + 1
                        h_prev = None if s == 0 else Hs[d_][:, :, t_prev, :]
                        for j in range(nch):
                            mch = mlo + j
                            if h_prev is not None:
                                for kc in range(4):
                                    nc.tensor.matmul(
                                        pt_[:, d_, j, :],
                                        wsl(wsb, d_, kc, mch),
                                        h_prev[:, kc, :],
                                        start=(kc == 0), stop=False)
                            nc.tensor.matmul(
                                pt_[:, d_, j, :], eye_sb[:],
                                xps[d_][:, mch, :],
                                start=(s == 0), stop=True)
                ga_g = work.tile([128, 2, 4, NB], dt.float32, tag="gag",
                                 bufs=2, name="gag")
                ga_ifo = work.tile([128, 2, 12, NB], dt.float32, tag="gaifo",
                                   bufs=3, name="gaifo")
                nc.scalar.activation(ga_g[:], pg_g[:], AF.Tanh, scale=DQ)
                nc.scalar.activation(ga_ifo[:], pg_ifo[:], AF.Sigmoid,
                                     scale=DQ)
                ig = work.tile([128, 2, 4, NB], dt.float32, tag="ig",
                               bufs=2, name="ig")
                nc.vector.tensor_tensor(out=ig[:], in0=ga_ifo[:, :, 0:4, :],
                                        in1=ga_g[:], op=ALU.mult)
                if s == 0:
                    nc.vector.tensor_copy(c_tile[:], ig[:])
                else:
                    nc.vector.tensor_tensor(out=c_tile[:], in0=c_tile[:],
                                            in1=ga_ifo[:, :, 4:8, :],
                                            op=ALU.mult)
                    nc.vector.tensor_tensor(out=c_tile[:], in0=c_tile[:],
                                            in1=ig[:], op=ALU.add)
                th = work.tile([128, 2, 4, NB], dt.float32, tag="th",
                               bufs=2, name="th")
                nc.scalar.activation(th[:], c_tile[:], AF.Tanh)
                for d_ in (0, 1):
                    t_log = s if d_ == 0 else T - 1 - s
                    eng = nc.vector if d_ == 0 else nc.gpsimd
                    eng.tensor_tensor(out=Hs[d_][:, :, t_log, :],
                                      in0=th[:, d_, :, :],
                                      in1=ga_ifo[:, d_, 8:12, :],
                                      op=ALU.mult)

            # ---------- layer-1 recurrence ----------
            c1 = work.tile([128, 2, 4, NB], dt.float32, tag="c1", bufs=1,
                           name="c1")
            for s in range(T):
                step_pair(whh1_sb, {
                    0: Xp1[0][:, :, :, s],
                    1: Xp1[1][:, :, :, T - 1 - s]}, H1, c1, s)

            # ---------- att1w = satt*Wenc^T enc  (fills L1-rec PE idle) ----
            for ac in range(4):
                for bblk in range(4):
                    pt = ps_mm.tile([128, 512], dt.float32, tag="pmm",
                                    name="pta1")
                    for bh in (0, 1):
                        b_ = 2 * bblk + bh
                        for ec in range(4):
                            nc.tensor.matmul(
                                pt[:, bh * 256:(bh + 1) * 256],
                                wencT_sb[:, ec, ac * 128:(ac + 1) * 128],
                                enc_ep_sb[:, b_, ec, :],
                                start=(ec == 0), stop=(ec == 3))
                    if (ac + bblk) % 2 == 0:
                        nc.vector.tensor_copy(
                            att1w[:, 2 * bblk:2 * bblk + 2, ac, :], pt[:])
                    else:
                        nc.scalar.copy(
                            att1w[:, 2 * bblk:2 * bblk + 2, ac, :], pt[:])

            # ---------- layer-2 input projections ----------
            for d_ in (0, 1):
                for mp in range(8):
                    pt = ps_mm.tile([128, 512], dt.float32, tag="pmm")
                    for half in (0, 1):
                        mch = 2 * mp + half
                        sl = pt[:, half * 128:(half + 1) * 128]
                        for kc in range(8):
                            rhs = (H1[0] if kc < 4 else H1[1])[:, kc % 4, :, :]
                            nc.tensor.matmul(
                                sl,
                                wsl(wih2_sb, d_, kc, mch),
                                rhs, start=(kc == 0), stop=False)
                        nc.tensor.matmul(
                            sl, b2row[d_][0:1, mch * 128:(mch + 1) * 128],
                            ones1b_sb[:], start=False, stop=True)
                    if mp % 2 == 0:
                        nc.vector.tensor_copy(
                            Xp2[d_][:, 2 * mp:2 * mp + 2, :, :]
                            .rearrange("p m t b -> p (m t b)"), pt[:, 0:256])
                    else:
                        nc.scalar.copy(
                            Xp2[d_][:, 2 * mp:2 * mp + 2, :, :]
                            .rearrange("p m t b -> p (m t b)"), pt[:, 0:256])

            # ---------- attention helpers ----------
            sc_t = ps_sc.tile([128, 2, 128], dt.float32, tag="sc",
                              name="scps")

            def emit_att2_pair(s):
                # both completed timesteps (15-s, s) in one 16-col matmul set
                lo, st = T - 1 - s, 2 * s - (T - 1)
                tsl = slice(lo, s + 1, st)
                pa2 = ps_g.tile([128, 4, 2, NB], dt.float32, tag="small",
                                bufs=2, name="pa2")
                for ac in range(4):
                    for kc in range(8):
                        nc.tensor.matmul(
                            pa2[:, ac, :, :],
                            wdecT_sb[:, kc, ac * 128:(ac + 1) * 128],
                            (H2[0] if kc < 4 else H2[1])[:, kc % 4, tsl, :],
                            start=(kc == 0), stop=False)
                    nc.tensor.matmul(
                        pa2[:, ac, :, :],
                        bea_row[0:1, ac * 128:(ac + 1) * 128],
                        ones1b_sb[0:1, 0:2 * NB].rearrange(
                            "o (u b) -> o u b", u=2),
                        start=False, stop=True)
                nc.vector.tensor_copy(att2pb_r[:, :, tsl, :], pa2[:])

            def emit_rw_col(b_, tt, engs):
                if isinstance(engs, int):
                    engs = (engs,) * 4
                col = b_ * T + tt
                rws = []
                for ac in range(4):
                    rw = rwp.tile([128, PP], dt.bfloat16, tag="rw")
                    eng = engs[ac]
                    if eng == 0:
                        nc.vector.tensor_scalar(
                            out=rw[:], in0=att1w[:, b_, ac, :],
                            scalar1=att2pb[:, ac, col:col + 1],
                            scalar2=0.0, op0=ALU.add, op1=ALU.max)
                    elif eng == 1:
                        nc.scalar.activation(
                            rw[:], att1w[:, b_, ac, :], AF.Relu,
                            bias=att2pb[:, ac, col:col + 1])
                    else:
                        nc.gpsimd.tensor_scalar(
                            out=rw[:], in0=att1w[:, b_, ac, :],
                            scalar1=att2pb[:, ac, col:col + 1],
                            scalar2=0.0, op0=ALU.add, op1=ALU.max)
                    rws.append(rw)
                # ph-major so the two accumulation groups in sc_t's single
                # psum zero-region never overlap (one must close before the
                # other starts)
                for ph in range(2):
                    for ac in range(4):
                        nc.tensor.matmul(
                            sc_t[:, ph, col:col + 1],
                            rws[ac][:, ph * 128:(ph + 1) * 128],
                            wrow_sb[:, ac:ac + 1],
                            start=(ac == 0), stop=(ac == 3))

            done_cols = set()

            # ---------- layer-2 recurrence (att2/relu interleaved) ----------
            c2 = work.tile([128, 2, 4, NB], dt.float32, tag="c2", bufs=1,
                           name="c2")
            for s in range(T):
                step_pair(whh2_sb, {
                    0: Xp2[0][:, :, s, :],
                    1: Xp2[1][:, :, T - 1 - s, :]}, H2, c2, s)
                if s >= 8:
                    emit_att2_pair(s)
                    # 3 columns/step interleaved, ops split across engines
                    # so no single engine's step budget is blown
                    for b_, tt, engs in ((0, s, (0, 0, 0, 0)),
                                         (0, T - 1 - s, (0, 0, 1, 2)),
                                         (1, s, (0, 0, 2, 2))):
                        emit_rw_col(b_, tt, engs)
                        done_cols.add((b_, tt))

            # hidden in (b,t)-column order for fc/gate stationary operands
            # (a stationary AP must have a single free dim, so the permuted
            # view must be materialized); SBUF->SBUF, Pool/DVE/ACT mix
            for kc in range(8):
                src_ = (H2[0] if kc < 4 else H2[1])[:, kc % 4, :, :] \
                    .rearrange("p t b -> p b t")
                if kc % 4 == 3:
                    nc.vector.tensor_copy(Hw[:, kc, :], src_)
                elif kc % 4 == 2:
                    nc.scalar.copy(Hw[:, kc, :], src_)
                else:
                    nc.gpsimd.tensor_copy(Hw[:, kc, :], src_)

            # switch the ACT table to exp_and_others (relu/tanh/copy stay
            # available in it; sigmoid is no longer needed)
            dmy = work.tile([1, 1], dt.float32, tag="dmy", bufs=1, name="dmy")
            nc.scalar.activation(dmy[:], ones1_sb[0:1, 0:1], AF.Exp)

            # ---------- tail: U-phase || deferred relu || per-b softmax ----
            # U matmuls, the deferred relu columns, and the per-batch
            # softmax/awe are emission-interleaved so PE (in-order) streams
            # the fc hidden part WHILE the elementwise engines chew relu.
            accD = accA = accP = 0.0   # engine-balance accumulators (ns)

            def emit_U(vb):
                nonlocal accA
                v0 = vb * 512
                vn = min(512, V - v0)
                wt = wtU[vb]
                pt = ps_mm.tile([128, 512], dt.float32, tag="pmm")
                for kc in range(8):
                    nc.tensor.matmul(pt[:, 0:vn], hid_cols(kc),
                                     wt[:, kc, 0:vn],
                                     start=(kc == 0), stop=False)
                nc.tensor.matmul(pt[:, 0:vn], ones1b_sb[:],
                                 bfcrow_sb[0:1, v0:v0 + vn],
                                 start=False, stop=True)
                nc.scalar.copy(U_sb[vb][:, 0:vn], pt[:, 0:vn])
                accA += 612.0

            def relu_col(b_, tt):
                nonlocal accD, accA, accP
                engs = []
                for _ in range(4):
                    costs = (accD + 127.0, accA + 398.0, accP + 544.0)
                    eng = int(np.argmin(costs))
                    if eng == 0:
                        accD = costs[0]
                    elif eng == 1:
                        accA = costs[1]
                    else:
                        accP = costs[2]
                    engs.append(eng)
                emit_rw_col(b_, tt, tuple(engs))

            def softmax_awe_b(b_):
                cs = slice(b_ * T, (b_ + 1) * T)
                nc.scalar.activation(E_sb[:, :, cs], sc_t[:, :, cs], AF.Exp,
                                     scale=DQ)
                sums = ps_g.tile([1, T], dt.float32, tag="small", bufs=2,
                                 name="sums")
                for ph in range(2):
                    nc.tensor.matmul(sums[:], ones_sb[:], E_sb[:, ph, cs],
                                     start=(ph == 0), stop=(ph == 1))
                nc.vector.reciprocal(recip_sb[0:1, cs], sums[:])
                bc = ps_g.tile([128, T], dt.float32, tag="small", bufs=2,
                               name="bc")
                nc.tensor.matmul(bc[:], ones1_sb[:], recip_sb[0:1, cs],
                                 start=True, stop=True)
                for ph in range(2):
                    nc.vector.tensor_tensor(out=alphaT[:, ph, cs],
                                            in0=E_sb[:, ph, cs],
                                            in1=bc[:], op=ALU.mult)
                pa = ps_g.tile([128, 4, T], dt.float32, tag="small", bufs=2,
                               name="pab")
                for ec in range(4):
                    for pc in range(2):
                        nc.tensor.matmul(
                            pa[:, ec, :],
                            enc_pe_sb[:, b_, pc, ec * 128:(ec + 1) * 128],
                            alphaT[:, pc, cs],
                            start=(pc == 0), stop=(pc == 1))
                nc.vector.tensor_copy(aweT[:, :, cs], pa[:])
                nonlocal accD, accA, accP
                accA += 170.0
                accD += 330.0
                accP += 250.0

            uq = list(range(NVB))

            def maybe_U(n=1):
                for _ in range(n):
                    if uq:
                        emit_U(uq.pop(0))

            softmax_awe_b(0)
            maybe_U(1)
            for b_ in range(1, NB):
                cnt = 0
                for tt in range(T):
                    if (b_, tt) in done_cols:
                        continue
                    relu_col(b_, tt)
                    cnt += 1
                    if cnt % 6 == 0:
                        maybe_U(1)
                softmax_awe_b(b_)
                maybe_U(1)
            maybe_U(len(uq))

            # ---------- gate: g01T = [g0, g1]/SCL ----------
            def fc_feat(kc):
                return hid_cols(kc) if kc < 8 else aweT[:, kc - 8, :]

            glT = ps_g.tile([128, 1], dt.float32, tag="small", bufs=2,
                            name="glt")
            for kc in range(12):
                nc.tensor.matmul(glT[:], fc_feat(kc),
                                 wdiff_sb[:, kc:kc + 1],
                                 start=(kc == 0), stop=False)
            nc.tensor.matmul(glT[:], ones1b_sb[:], bdiff_sb[:],
                             start=False, stop=True)
            gex = work.tile([128, 1], dt.float32, tag="gex", bufs=1,
                            name="gex")
            nc.scalar.activation(gex[:], glT[:], AF.Exp, scale=-1.0)
            # g0/SCL = 1/((1+gex)*SCL); g1/SCL = 1/SCL - g0/SCL
            nc.vector.tensor_scalar(out=gex[:], in0=gex[:], scalar1=1.0,
                                    scalar2=SCL, op0=ALU.add, op1=ALU.mult)
            nc.vector.reciprocal(g01T[:, 0:1], gex[:])
            nc.vector.tensor_scalar(out=g01T[:, 1:2], in0=g01T[:, 0:1],
                                    scalar1=-1.0, scalar2=DQ,
                                    op0=ALU.mult, op1=ALU.add)

            # ---------- fc awe-part + gate combine + writeback (bf16) ------
            for vb in range(NVB):
                v0 = vb * 512
                vn = min(512, V - v0)
                wt = wtA[vb]
                pt = ps_mm.tile([128, 512], dt.float32, tag="pmm")
                for kc in range(4):
                    nc.tensor.matmul(pt[:, 0:vn], aweT[:, kc, :],
                                     wt[:, kc, 0:vn],
                                     start=(kc == 0), stop=False)
                nc.tensor.matmul(pt[:, 0:vn], ones1b_sb[:],
                                 bfcrow_sb[0:1, v0:v0 + vn],
                                 start=False, stop=True)
                ost = outp.tile([128, 512], dt.bfloat16, tag="ost")
                nc.scalar.activation(ost[:, 0:vn], U_sb[vb][:, 0:vn],
                                     AF.Identity, scale=g01T[:, 0:1])
                nc.vector.scalar_tensor_tensor(
                    out=ost[:, 0:vn], in0=pt[:, 0:vn],
                    scalar=g01T[:, 1:2], in1=ost[:, 0:vn],
                    op0=ALU.mult, op1=ALU.add)
                dst = bass.AP(tensor=out_t[:].tensor, offset=v0,
                              ap=[[V, 128], [1, vn]])
                # alternate ACT/Pool DMA queues: off the head-blocked sync
                # queue, and neither queue eats all the serialization
                if vb % 2 == 0:
                    nc.gpsimd.dma_start(out=dst, in_=ost[:, 0:vn])
                else:
                    nc.scalar.dma_start(out=dst, in_=ost[:, 0:vn])

    nc.compile()
    return nc


def _host_prep(inputs):
    f32 = np.float32

    def bf(x):
        return np.ascontiguousarray(np.asarray(x, f32).astype(BF))

    def q8(x):
        # fixed power-of-2 scale + clip; RNE via astype
        y = np.clip(np.asarray(x, f32) * SCL, -E3MAX, E3MAX)
        return np.ascontiguousarray(y.astype(E3))

    enc = np.asarray(inputs["encoder_out"], f32)
    enc_p = enc.reshape(B, E, PP)
    feats_all = enc.mean(axis=2)                  # (B, E, W=T)

    # permute gate blocks (i,f,g,o) -> (i,f,o,g) so one sigmoid spans i,f,o
    gp = np.r_[0:2 * D, 3 * D:4 * D, 2 * D:3 * D]

    common = {}
    common["wih1f"] = q8(np.asarray(inputs["Wih1"], f32).T[:, gp])
    common["wih1r"] = q8(np.asarray(inputs["Wih1r"], f32).T[:, gp])
    common["whh1f"] = q8(np.asarray(inputs["Whh1"], f32).T[:, gp])
    common["whh1r"] = q8(np.asarray(inputs["Whh1r"], f32).T[:, gp])
    common["wih2f"] = q8(np.asarray(inputs["Wih2"], f32).T[:, gp])
    common["wih2r"] = q8(np.asarray(inputs["Wih2r"], f32).T[:, gp])
    common["whh2f"] = q8(np.asarray(inputs["Whh2"], f32).T[:, gp])
    common["whh2r"] = q8(np.asarray(inputs["Whh2r"], f32).T[:, gp])
    common["b1f"] = bf(np.asarray(inputs["bih1"] + inputs["bhh1"],
                                  f32)[gp] * SCL)
    common["b1r"] = bf(np.asarray(inputs["bih1r"] + inputs["bhh1r"],
                                  f32)[gp] * SCL)
    common["b2f"] = bf(np.asarray(inputs["bih2"] + inputs["bhh2"],
                                  f32)[gp] * SCL)
    common["b2r"] = bf(np.asarray(inputs["bih2r"] + inputs["bhh2r"],
                                  f32)[gp] * SCL)
    common["wencT"] = q8(np.asarray(inputs["Wenc"], f32).T)
    common["wdecT"] = q8(np.asarray(inputs["Wdec"], f32).T)
    common["beab"] = bf(np.asarray(inputs["benc"] + inputs["bdec"],
                                   f32) * SCL)
    common["wfullb"] = bf(np.asarray(inputs["Wfull"], f32)[0])
    wg = np.asarray(inputs["Wg"], f32)
    common["wdiffT"] = bf(wg[0] - wg[1])
    bd = float(np.asarray(inputs["bg"], f32)[0]
               - np.asarray(inputs["bg"], f32)[1])
    common["bdiffb"] = bf(np.array([[bd]], f32))
    common["eye128"] = bf(np.eye(128, dtype=f32))
    common["wfcT"] = q8(np.asarray(inputs["Wfc"], f32).T)
    bfc = np.zeros(VCH * 128, f32)
    bfc[:V] = np.asarray(inputs["bfc"], f32)
    common["bfcp"] = bf(bfc * SCL)

    in_maps = []
    for c in range(NCORES):
        m = dict(common)
        sl = enc_p[c * NB:(c + 1) * NB]
        m["enc_ep"] = bf(sl)
        m["enc_pe"] = bf(np.ascontiguousarray(sl.transpose(0, 2, 1)))
        m["featsb"] = bf(np.ascontiguousarray(
            feats_all[c * NB:(c + 1) * NB].transpose(1, 0, 2)))
        in_maps.append(m)
    return in_maps


def _get_program():
    if "nc" not in _prog_cache:
        _prog_cache["nc"] = _build_program()
    return _prog_cache["nc"]


def kernel(**inputs):
    from concourse.bass_utils import run_bass_kernel_spmd

    nc = _get_program()
    in_maps = _host_prep(inputs)
    res = run_bass_kernel_spmd(nc, in_maps, list(range(NCORES)))
    # per-core result is [b, t, v] bf16; assemble to (T, B, V) f32
    out = np.concatenate(
        [np.asarray(res.results[c]["out"], np.float32).transpose(1, 0, 2)
         for c in range(NCORES)],
        axis=1)
    return np.ascontiguousarray(out, np.float32)


# revision 30
# speedup vs baseline: 1.1590x; 1.0110x over previous
"""Trainium2 Bass kernel for DecoderWithAttention (bidirectional 2-layer LSTM +
additive attention + gated fc), data-parallel over batch across 8 NeuronCores.

Shapes (hardcoded): encoder_out (64, 512, 16, 16), T=16, D=A=512, V=5000.
Per core: 8 batches, full network, weights replicated (no collectives under
this axon terminal, so each core is fully independent).

Major layout/optimization decisions (v2):
  - All large weights (Wih*/Whh*/Wenc/Wdec/Wfc) are stored+streamed as
    float8 e3m4, host-scaled by a fixed power-of-2 (SCL=32, clipped) so the
    fp8 mantissa window is centered; dequant is folded into ACT scale params
    (gates: sigmoid/tanh(psum/SCL); softmax: exp(sc/SCL); fc: 1/SCL folded
    into the gate coefficients g0/g1).  Halves the ~43MB/core DMA to ~20MB.
  - feats (AdaptiveAvgPool over H) is computed host-side (input prep),
    removing the on-chip reduce and letting L1 start ~10us earlier.
  - LSTM step: psum groups (g | ifo) -> 3 ACT ops/step (tanh_g, sig_ifo,
    tanh_c); both directions fused in each op; h written by DVE (fwd) and
    Pool (rev).  Xp psum->SBUF copies ride on Pool/DVE.
  - Weights pinned in SBUF (no rotation pool), so every weight DMA issues
    back-to-back with no WAR stalls; wfc hidden/awe parts stream through
    small fp8 rotation pools ordered so the sync queue never head-blocks.
  - Attention: att1w=Wenc^T enc precomputed (scheduler lifts it into L1-rec
    PE idle); att2 per-timestep inside the L2 loop; relu/score columns for
    b0/b1 interleaved into the L2 loop (DVE/ACT/Pool), the rest deferred and
    engine-balanced; softmax+awe pipelined per-batch behind the relu columns.
  - fc factorized around the gate (g0*(hid Wh+bfc) + g1*(awe Wa+bfc)); the
    hidden part streams right after the L2 recurrence; gate sigmoid folds
    1/SCL; output written bf16 (host upcasts to f32).
"""

import numpy as np
import ml_dtypes

BF = ml_dtypes.bfloat16
E3 = ml_dtypes.float8_e3m4
B, E, HH, WW = 64, 512, 16, 16
T = WW          # 16 timesteps
PP = HH * WW    # 256 attention positions
D = 512
A = 512
V = 5000
G = 4 * D
NB = 8          # batches per core
NCORES = 8
F = 2 * D + E   # 1536
VCH = (V + 127) // 128  # 40
NVB = 10        # fc v-blocks of 512

SCL = 32.0      # fixed power-of-2 fp8 scale for all quantized weights
DQ = 1.0 / SCL  # dequant factor (exact)
E3MAX = 15.5

_prog_cache = {}


def _build_program():
    import concourse.bass as bass
    import concourse.bacc as bacc
    import concourse.mybir as mybir
    import concourse.tile as tile

    dt = mybir.dt
    AF = mybir.ActivationFunctionType
    ALU = mybir.AluOpType

    nc = bacc.Bacc("TRN2", target_bir_lowering=False, debug=False,
                   num_devices=NCORES, dynamic_dma_scratch_size=2048)

    def din(name, shape, d=dt.bfloat16):
        return nc.dram_tensor(name, shape, d, kind="ExternalInput")

    f8 = dt.float8e3
    featsb = din("featsb", [E, NB, T])            # host mean over H, bf16
    enc_ep = din("enc_ep", [NB, E, PP])           # [b, e, p]
    enc_pe = din("enc_pe", [NB, PP, E])           # [b, p, e]
    wih1 = {0: din("wih1f", [E, G], f8), 1: din("wih1r", [E, G], f8)}
    whh1 = {0: din("whh1f", [D, G], f8), 1: din("whh1r", [D, G], f8)}
    wih2 = {0: din("wih2f", [2 * D, G], f8), 1: din("wih2r", [2 * D, G], f8)}
    whh2 = {0: din("whh2f", [D, G], f8), 1: din("whh2r", [D, G], f8)}
    b1 = {0: din("b1f", [G]), 1: din("b1r", [G])}   # (bih+bhh)*SCL bf16
    b2 = {0: din("b2f", [G]), 1: din("b2r", [G])}
    wencT = din("wencT", [E, A], f8)
    wdecT = din("wdecT", [2 * D, A], f8)
    beab = din("beab", [A])                       # (benc+bdec)*SCL, bf16
    wfullb = din("wfullb", [A])                   # Wfull[0] bf16 (signed)
    wdiffT = din("wdiffT", [F])                   # Wg[0]-Wg[1] bf16
    bdiffb = din("bdiffb", [1, 1])                # bg0-bg1 bf16
    eye128 = din("eye128", [128, 128])
    wfcT = din("wfcT", [F, V], f8)
    bfcp = din("bfcp", [VCH * 128])               # bfc*SCL, bf16 padded
    out_t = nc.dram_tensor("out", [NB, T, V], dt.bfloat16,
                           kind="ExternalOutput")

    with tile.TileContext(nc) as tc:
        with (
            tc.tile_pool(name="const", bufs=1) as const,
            tc.tile_pool(name="work", bufs=4) as work,
            tc.tile_pool(name="rwp", bufs=10) as rwp,
            tc.tile_pool(name="wfcp", bufs=2) as wfcp,
            tc.tile_pool(name="outp", bufs=3) as outp,
            tc.tile_pool(name="ps_g", bufs=1, space="PSUM") as ps_g,
            tc.tile_pool(name="ps_mm", bufs=3, space="PSUM") as ps_mm,
            tc.tile_pool(name="ps_sc", bufs=1, space="PSUM") as ps_sc,
        ):
            dma = nc.sync.dma_start

            # ---------------- DMA section (sync-queue program order) -------
            # tiny tiles the early phases depend on go first
            feats = const.tile([128, 4, NB, T], dt.bfloat16)   # (ech, b, t)
            dma(out=feats[:],
                in_=featsb[:].rearrange("(ec ep) b t -> ep ec b t", ep=128))
            eye_sb = const.tile([128, 128], dt.bfloat16)
            dma(out=eye_sb[:], in_=eye128[:])
            b1row, b2row = {}, {}
            for d_ in (0, 1):
                b1row[d_] = const.tile([1, G], dt.bfloat16, tag=f"b1r_{d_}",
                                       bufs=1, name=f"b1row{d_}")
                dma(out=b1row[d_][:], in_=b1[d_][:])

            # pinned fp8 LSTM weights.  L1 weights are split into two
            # half-G tiles each so the first projection / recurrence matmuls
            # only wait on the first half-MB DMA.
            def wload(dram, kchunks, nm, splits=1):
                gs = G // splits
                ts = []
                for i in range(splits):
                    t_ = const.tile([128, kchunks, gs], f8, name=f"{nm}_{i}")
                    dma(out=t_[:],
                        in_=dram[:, i * gs:(i + 1) * gs]
                        .rearrange("(kc kp) g -> kp kc g", kp=128))
                    ts.append(t_)
                return ts

            def wsl(wsb, d_, kc, mch):
                # weight slice [128, 128] for gate chunk mch
                ts = wsb[d_]
                n = len(ts)
                per = 16 // n
                t_ = ts[mch // per]
                j = mch % per
                return t_[:, kc, j * 128:(j + 1) * 128]

            wih1_sb = {d_: wload(wih1[d_], 4, f"wih1_{d_}", 2)
                       for d_ in (0, 1)}
            whh1_sb = {d_: wload(whh1[d_], 4, f"whh1_{d_}", 2)
                       for d_ in (0, 1)}
            wencT_sb = const.tile([128, 4, A], f8)            # (ech, a)
            dma(out=wencT_sb[:],
                in_=wencT[:].rearrange("(ec ep) a -> ep ec a", ep=128))
            enc_ep_sb = const.tile([128, NB, 4, PP], dt.bfloat16,
                                   tag="encbuf", bufs=1, name="enc_ep")
            for bh in (0, 1):
                dma(out=enc_ep_sb[:, 4 * bh:4 * bh + 4, :, :],
                    in_=enc_ep[4 * bh:4 * bh + 4]
                    .rearrange("b (ec ep) p -> ep b ec p", ep=128))
            for d_ in (0, 1):
                b2row[d_] = const.tile([1, G], dt.bfloat16, tag=f"b2r_{d_}",
                                       name=f"b2row{d_}")
                dma(out=b2row[d_][:], in_=b2[d_][:])
            wih2_sb = {d_: wload(wih2[d_], 8, f"wih2_{d_}") for d_ in (0, 1)}
            whh2_sb = {d_: wload(whh2[d_], 4, f"whh2_{d_}") for d_ in (0, 1)}
            # prime the tanh/sigmoid ACT table during the DMA head so the
            # first gate activation doesn't eat a 1.3us table load
            prime = work.tile([1, 1], dt.float32, tag="dmy", bufs=1,
                              name="prime")
            nc.scalar.activation(prime[:], eye_sb[0:1, 0:1], AF.Tanh)
            wdecT_sb = const.tile([128, 8, A], f8)            # (kch, a)
            dma(out=wdecT_sb[:],
                in_=wdecT[:].rearrange("(kc kp) a -> kp kc a", kp=128))
            wrow_sb = const.tile([128, 4], dt.bfloat16)       # Wfull (signed)
            dma(out=wrow_sb[:], in_=wfullb[:].rearrange("(c p) -> p c", p=128))
            bea_row = const.tile([1, A], dt.bfloat16)
            dma(out=bea_row[:], in_=beab[:])
            wdiff_sb = const.tile([128, 12], dt.bfloat16)
            dma(out=wdiff_sb[:], in_=wdiffT[:].rearrange("(c p) -> p c", p=128))
            bdiff_sb = const.tile([1, 1], dt.bfloat16)
            dma(out=bdiff_sb[:], in_=bdiffb[:])
            bfcrow_sb = const.tile([1, VCH * 128], dt.bfloat16)
            dma(out=bfcrow_sb[:], in_=bfcp[:])
            # enc_pe shares the enc_ep slot (same 16KB); its DMA waits until
            # the att1w matmuls (the only enc_ep readers) are done
            enc_pe_sb = const.tile([128, NB, 2, E], dt.bfloat16,
                                   tag="encbuf", name="enc_pe")
            dma(out=enc_pe_sb[:],
                in_=enc_pe[:].rearrange("b (pc pp) e -> pp b pc e", pp=128))

            # fc weight streams (fp8). wfcU rotation: first 6 issue during
            # the recurrences, the rest pace with the U matmuls.  wfcA's
            # first 4 issue early; the rest pace with the A matmuls.  Order
            # keeps sync-queue head-blocking monotone with need times.
            wtU, wtA = [], []

            def load_U(vb):
                wt = wfcp.tile([128, 8, 512], f8, tag="wfcU", bufs=5,
                               name="wtU")
                v0 = vb * 512
                vn = min(512, V - v0)
                dma(out=wt[:, :, 0:vn],
                    in_=wfcT[0:2 * D, v0:v0 + vn]
                    .rearrange("(kc kp) v -> kp kc v", kp=128))
                wtU.append(wt)

            def load_A(vb):
                wt = wfcp.tile([128, 4, 512], f8, tag="wfcA", bufs=2,
                               name="wtA")
                v0 = vb * 512
                vn = min(512, V - v0)
                dma(out=wt[:, :, 0:vn],
                    in_=wfcT[2 * D:F, v0:v0 + vn]
                    .rearrange("(kc kp) v -> kp kc v", kp=128))
                wtA.append(wt)

            for vb in range(5):
                load_U(vb)
            for vb in range(2):
                load_A(vb)

            def load_A_slot(vb, tag, nm):
                wt = const.tile([128, 4, 512], f8, tag=tag, name=nm)
                v0 = vb * 512
                vn = min(512, V - v0)
                dma(out=wt[:, :, 0:vn],
                    in_=wfcT[2 * D:F, v0:v0 + vn]
                    .rearrange("(kc kp) v -> kp kc v", kp=128))
                wtA.append(wt)

            # ---------------- persistent SBUF state ----------------
            Xp1 = {d_: const.tile([128, 16, NB, T], dt.bfloat16,
                                  tag=f"xp1_{d_}", bufs=1, name=f"Xp1_{d_}")
                   for d_ in (0, 1)}                          # (gch, b, t)
            Xp2 = {d_: const.tile([128, 16, T, NB], dt.bfloat16,
                                  tag=f"xp2_{d_}", bufs=1, name=f"Xp2_{d_}")
                   for d_ in (0, 1)}                          # (gch, t, b)
            H1 = {d_: const.tile([128, 4, T, NB], dt.bfloat16,
                                 tag=f"h1_{d_}", name=f"H1_{d_}")
                  for d_ in (0, 1)}                           # (dch, t, b)
            H2 = {d_: const.tile([128, 4, T, NB], dt.bfloat16,
                                 tag=f"h2_{d_}", name=f"H2_{d_}")
                  for d_ in (0, 1)}
            att1w = const.tile([128, NB, 4, PP], dt.bfloat16)  # (b, ach, p)
            att2pb = const.tile([128, 4, 128], dt.float32)     # (ach, (b,t))
            att2pb_r = att2pb[:].rearrange("p a (b t) -> p a t b", t=T)
            E_sb = const.tile([128, 2, 128], dt.bfloat16)      # exp(sc/SCL)
            alphaT = const.tile([128, 2, 128], dt.bfloat16)    # (pch, (b,t))
            aweT = const.tile([128, 4, 128], dt.bfloat16)      # (ech, (b,t))
            recip_sb = const.tile([1, 128], dt.float32)
            U_sb = [const.tile([128, 512], dt.bfloat16, tag=f"u{vb}",
                               name=f"U{vb}") for vb in range(NVB)]
            g01T = const.tile([128, 2], dt.float32)
            ones1_sb = const.tile([1, 128], dt.float32)
            nc.vector.memset(ones1_sb[:], 1.0)
            ones1b_sb = const.tile([1, 128], dt.bfloat16)
            nc.vector.memset(ones1b_sb[:], 1.0)
            ones_sb = const.tile([128, 1], dt.bfloat16)
            nc.vector.memset(ones_sb[:], 1.0)

            Hw = const.tile([128, 8, 128], dt.bfloat16)  # (kch, (b,t))

            def hid_cols(kc):
                return Hw[:, kc, :]

            # ---------- layer-1 input projections (all t, N=128) ----------
            for d_ in (0, 1):
                for mp in range(8):
                    pt = ps_mm.tile([128, 512], dt.float32, tag="pmm")
                    for half in (0, 1):
                        mch = 2 * mp + half
                        sl = pt[:, half * 128:(half + 1) * 128]
                        for kc in range(4):
                            nc.tensor.matmul(
                                sl,
                                wsl(wih1_sb, d_, kc, mch),
                                feats[:, kc, :, :], start=(kc == 0),
                                stop=False)
                        nc.tensor.matmul(
                            sl, b1row[d_][0:1, mch * 128:(mch + 1) * 128],
                            ones1b_sb[:], start=False, stop=True)
                    # gpsimd cannot read PSUM on hw; alternate DVE/ACT
                    if mp % 2 == 0:
                        nc.vector.tensor_copy(
                            Xp1[d_][:, 2 * mp:2 * mp + 2, :, :]
                            .rearrange("p m b w -> p (m b w)"), pt[:, 0:256])
                    else:
                        nc.scalar.copy(
                            Xp1[d_][:, 2 * mp:2 * mp + 2, :, :]
                            .rearrange("p m b w -> p (m b w)"), pt[:, 0:256])

            # ---------- LSTM fused step pair ----------
            # Gate blocks host-permuted to (i, f, o, g):
            # ch 0-3=i, 4-7=f, 8-11=o, 12-15=g.
            # psum groups: g (tanh feeds ig first) | ifo (single sigmoid).
            def step_pair(wsb, xps, Hs, c_tile, s):
                pg_g = ps_g.tile([128, 2, 4, NB], dt.float32, tag="pgg",
                                 bufs=1, name="pgg")
                pg_ifo = ps_g.tile([128, 2, 12, NB], dt.float32, tag="pgifo",
                                   bufs=1, name="pgifo")
                for pt_, mlo, nch in ((pg_g, 12, 4), (pg_ifo, 0, 12)):
                    for d_ in (0, 1):
                        t_log = s if d_ == 0 else T - 1 - s
                        t_prev = t_log - 1 if d_ == 0 else t_log + 1
                        h_prev = None if s == 0 else Hs[d_][:, :, t_prev, :]
                        for j in range(nch):
                            mch = mlo + j
                            if h_prev is not None:
                                for kc in range(4):
                                    nc.tensor.matmul(
                                        pt_[:, d_, j, :],
                                        wsl(wsb, d_, kc, mch),
                                        h_prev[:, kc, :],
                                        start=(kc == 0), stop=False)
                            nc.tensor.matmul(
                                pt_[:, d_, j, :], eye_sb[:],
                                xps[d_][:, mch, :],
                                start=(s == 0), stop=True)
                ga_g = work.tile([128, 2, 4, NB], dt.float32, tag="gag",
                                 bufs=2, name="gag")
                ga_ifo = work.tile([128, 2, 12, NB], dt.float32, tag="gaifo",
                                   bufs=3, name="gaifo")
                nc.scalar.activation(ga_g[:], pg_g[:], AF.Tanh, scale=DQ)
                nc.scalar.activation(ga_ifo[:], pg_ifo[:], AF.Sigmoid,
                                     scale=DQ)
                ig = work.tile([128, 2, 4, NB], dt.float32, tag="ig",
                               bufs=2, name="ig")
                nc.vector.tensor_tensor(out=ig[:], in0=ga_ifo[:, :, 0:4, :],
                                        in1=ga_g[:], op=ALU.mult)
                if s == 0:
                    nc.vector.tensor_copy(c_tile[:], ig[:])
                else:
                    nc.vector.tensor_tensor(out=c_tile[:], in0=c_tile[:],
                                            in1=ga_ifo[:, :, 4:8, :],
                                            op=ALU.mult)
                    nc.vector.tensor_tensor(out=c_tile[:], in0=c_tile[:],
                                            in1=ig[:], op=ALU.add)
                th = work.tile([128, 2, 4, NB], dt.float32, tag="th",
                               bufs=2, name="th")
                nc.scalar.activation(th[:], c_tile[:], AF.Tanh)
                for d_ in (0, 1):
                    t_log = s if d_ == 0 else T - 1 - s
                    eng = nc.vector if d_ == 0 else nc.gpsimd
                    eng.tensor_tensor(out=Hs[d_][:, :, t_log, :],
                                      in0=th[:, d_, :, :],
                                      in1=ga_ifo[:, d_, 8:12, :],
                                      op=ALU.mult)

            # ---------- layer-1 recurrence ----------
            c1 = work.tile([128, 2, 4, NB], dt.float32, tag="c1", bufs=1,
                           name="c1")
            for s in range(T):
                step_pair(whh1_sb, {
                    0: Xp1[0][:, :, :, s],
                    1: Xp1[1][:, :, :, T - 1 - s]}, H1, c1, s)

            # awe-part weights vb2-3 ride the retired Xp1 slots (their
            # L1-recurrence readers are all emitted above)
            load_A_slot(2, "xp1_0", "wtAx0")
            load_A_slot(3, "xp1_1", "wtAx1")

            # ---------- att1w = satt*Wenc^T enc  (fills L1-rec PE idle) ----
            for ac in range(4):
                for bblk in range(4):
                    pt = ps_mm.tile([128, 512], dt.float32, tag="pmm",
                                    name="pta1")
                    for bh in (0, 1):
                        b_ = 2 * bblk + bh
                        for ec in range(4):
                            nc.tensor.matmul(
                                pt[:, bh * 256:(bh + 1) * 256],
                                wencT_sb[:, ec, ac * 128:(ac + 1) * 128],
                                enc_ep_sb[:, b_, ec, :],
                                start=(ec == 0), stop=(ec == 3))
                    if (ac + bblk) % 2 == 0:
                        nc.vector.tensor_copy(
                            att1w[:, 2 * bblk:2 * bblk + 2, ac, :], pt[:])
                    else:
                        nc.scalar.copy(
                            att1w[:, 2 * bblk:2 * bblk + 2, ac, :], pt[:])

            # ---------- layer-2 input projections ----------
            for d_ in (0, 1):
                for mp in range(8):
                    pt = ps_mm.tile([128, 512], dt.float32, tag="pmm")
                    for half in (0, 1):
                        mch = 2 * mp + half
                        sl = pt[:, half * 128:(half + 1) * 128]
                        for kc in range(8):
                            rhs = (H1[0] if kc < 4 else H1[1])[:, kc % 4, :, :]
                            nc.tensor.matmul(
                                sl,
                                wsl(wih2_sb, d_, kc, mch),
                                rhs, start=(kc == 0), stop=False)
                        nc.tensor.matmul(
                            sl, b2row[d_][0:1, mch * 128:(mch + 1) * 128],
                            ones1b_sb[:], start=False, stop=True)
                    if mp % 2 == 0:
                        nc.vector.tensor_copy(
                            Xp2[d_][:, 2 * mp:2 * mp + 2, :, :]
                            .rearrange("p m t b -> p (m t b)"), pt[:, 0:256])
                    else:
                        nc.scalar.copy(
                            Xp2[d_][:, 2 * mp:2 * mp + 2, :, :]
                            .rearrange("p m t b -> p (m t b)"), pt[:, 0:256])

            # ---------- attention helpers ----------
            sc_t = ps_sc.tile([128, 2, 128], dt.float32, tag="sc",
                              name="scps")

            def emit_att2_pair(s):
                # both completed timesteps (15-s, s) in one 16-col matmul set
                lo, st = T - 1 - s, 2 * s - (T - 1)
                tsl = slice(lo, s + 1, st)
                pa2 = ps_g.tile([128, 4, 2, NB], dt.float32, tag="small",
                                bufs=2, name="pa2")
                for ac in range(4):
                    for kc in range(8):
                        nc.tensor.matmul(
                            pa2[:, ac, :, :],
                            wdecT_sb[:, kc, ac * 128:(ac + 1) * 128],
                            (H2[0] if kc < 4 else H2[1])[:, kc % 4, tsl, :],
                            start=(kc == 0), stop=False)
                    nc.tensor.matmul(
                        pa2[:, ac, :, :],
                        bea_row[0:1, ac * 128:(ac + 1) * 128],
                        ones1b_sb[0:1, 0:2 * NB].rearrange(
                            "o (u b) -> o u b", u=2),
                        start=False, stop=True)
                nc.vector.tensor_copy(att2pb_r[:, :, tsl, :], pa2[:])

            def emit_rw_col(b_, tt, engs):
                if isinstance(engs, int):
                    engs = (engs,) * 4
                col = b_ * T + tt
                rws = []
                for ac in range(4):
                    rw = rwp.tile([128, PP], dt.bfloat16, tag="rw")
                    eng = engs[ac]
                    if eng == 0:
                        nc.vector.tensor_scalar(
                            out=rw[:], in0=att1w[:, b_, ac, :],
                            scalar1=att2pb[:, ac, col:col + 1],
                            scalar2=0.0, op0=ALU.add, op1=ALU.max)
                    elif eng == 1:
                        nc.scalar.activation(
                            rw[:], att1w[:, b_, ac, :], AF.Relu,
                            bias=att2pb[:, ac, col:col + 1])
                    else:
                        nc.gpsimd.tensor_scalar(
                            out=rw[:], in0=att1w[:, b_, ac, :],
                            scalar1=att2pb[:, ac, col:col + 1],
                            scalar2=0.0, op0=ALU.add, op1=ALU.max)
                    rws.append(rw)
                # ph-major so the two accumulation groups in sc_t's single
                # psum zero-region never overlap (one must close before the
                # other starts)
                for ph in range(2):
                    for ac in range(4):
                        nc.tensor.matmul(
                            sc_t[:, ph, col:col + 1],
                            rws[ac][:, ph * 128:(ph + 1) * 128],
                            wrow_sb[:, ac:ac + 1],
                            start=(ac == 0), stop=(ac == 3))

            done_cols = set()

            # ---------- layer-2 recurrence (att2/relu interleaved) ----------
            c2 = work.tile([128, 2, 4, NB], dt.float32, tag="c2", bufs=1,
                           name="c2")
            for s in range(T):
                step_pair(whh2_sb, {
                    0: Xp2[0][:, :, s, :],
                    1: Xp2[1][:, :, T - 1 - s, :]}, H2, c2, s)
                if s >= 8:
                    emit_att2_pair(s)
                    # 3 columns/step interleaved, ops split across engines
                    # so no single engine's step budget is blown
                    for b_, tt, engs in ((0, s, (0, 0, 0, 0)),
                                         (0, T - 1 - s, (0, 0, 1, 2)),
                                         (1, s, (0, 0, 2, 2))):
                        emit_rw_col(b_, tt, engs)
                        done_cols.add((b_, tt))

            # remaining fc weight streams: wfcU 5-9 pace with the U
            # matmuls; wfcA 4-5 ride the retired Xp2 slots; 6-9 rotate the
            # wfcA pool behind the A matmuls
            for vb in range(5, NVB):
                load_U(vb)
            load_A_slot(4, "xp2_0", "wtAx2")
            load_A_slot(5, "xp2_1", "wtAx3")
            for vb in range(6, NVB):
                load_A(vb)

            # hidden in (b,t)-column order for fc/gate stationary operands
            # (a stationary AP must have a single free dim, so the permuted
            # view must be materialized); SBUF->SBUF, Pool/DVE/ACT mix
            for kc in range(8):
                src_ = (H2[0] if kc < 4 else H2[1])[:, kc % 4, :, :] \
                    .rearrange("p t b -> p b t")
                if kc % 4 == 3:
                    nc.vector.tensor_copy(Hw[:, kc, :], src_)
                elif kc % 4 == 2:
                    nc.scalar.copy(Hw[:, kc, :], src_)
                else:
                    nc.gpsimd.tensor_copy(Hw[:, kc, :], src_)

            # switch the ACT table to exp_and_others (relu/tanh/copy stay
            # available in it; sigmoid is no longer needed)
            dmy = work.tile([1, 1], dt.float32, tag="dmy", bufs=1, name="dmy")
            nc.scalar.activation(dmy[:], ones1_sb[0:1, 0:1], AF.Exp)

            # ---------- tail: U-phase || deferred relu || per-b softmax ----
            # U matmuls, the deferred relu columns, and the per-batch
            # softmax/awe are emission-interleaved so PE (in-order) streams
            # the fc hidden part WHILE the elementwise engines chew relu.
            accD = accA = accP = 0.0   # engine-balance accumulators (ns)

            def emit_U(vb):
                nonlocal accA
                v0 = vb * 512
                vn = min(512, V - v0)
                wt = wtU[vb]
                pt = ps_mm.tile([128, 512], dt.float32, tag="pmm")
                for kc in range(8):
                    nc.tensor.matmul(pt[:, 0:vn], hid_cols(kc),
                                     wt[:, kc, 0:vn],
                                     start=(kc == 0), stop=False)
                nc.tensor.matmul(pt[:, 0:vn], ones1b_sb[:],
                                 bfcrow_sb[0:1, v0:v0 + vn],
                                 start=False, stop=True)
                nc.scalar.copy(U_sb[vb][:, 0:vn], pt[:, 0:vn])
                accA += 612.0

            def relu_col(b_, tt):
                nonlocal accD, accA, accP
                engs = []
                for _ in range(4):
                    costs = (accD + 127.0, accA + 398.0, accP + 544.0)
                    eng = int(np.argmin(costs))
                    if eng == 0:
                        accD = costs[0]
                    elif eng == 1:
                        accA = costs[1]
                    else:
                        accP = costs[2]
                    engs.append(eng)
                emit_rw_col(b_, tt, tuple(engs))

            def softmax_awe_b(b_):
                cs = slice(b_ * T, (b_ + 1) * T)
                nc.scalar.activation(E_sb[:, :, cs], sc_t[:, :, cs], AF.Exp,
                                     scale=DQ)
                sums = ps_g.tile([1, T], dt.float32, tag="small", bufs=2,
                                 name="sums")
                for ph in range(2):
                    nc.tensor.matmul(sums[:], ones_sb[:], E_sb[:, ph, cs],
                                     start=(ph == 0), stop=(ph == 1))
                nc.vector.reciprocal(recip_sb[0:1, cs], sums[:])
                bc = ps_g.tile([128, T], dt.float32, tag="small", bufs=2,
                               name="bc")
                nc.tensor.matmul(bc[:], ones1_sb[:], recip_sb[0:1, cs],
                                 start=True, stop=True)
                for ph in range(2):
                    nc.vector.tensor_tensor(out=alphaT[:, ph, cs],
                                            in0=E_sb[:, ph, cs],
                                            in1=bc[:], op=ALU.mult)
                pa = ps_g.tile([128, 4, T], dt.float32, tag="small", bufs=2,
                               name="pab")
                for ec in range(4):
                    for pc in range(2):
                        nc.tensor.matmul(
                            pa[:, ec, :],
                            enc_pe_sb[:, b_, pc, ec * 128:(ec + 1) * 128],
                            alphaT[:, pc, cs],
                            start=(pc == 0), stop=(pc == 1))
                nc.vector.tensor_copy(aweT[:, :, cs], pa[:])
                nonlocal accD, accA, accP
                accA += 170.0
                accD += 330.0
                accP += 250.0

            uq = list(range(NVB))

            def maybe_U(n=1):
                for _ in range(n):
                    if uq:
                        emit_U(uq.pop(0))

            softmax_awe_b(0)
            maybe_U(1)
            for b_ in range(1, NB):
                cnt = 0
                for tt in range(T):
                    if (b_, tt) in done_cols:
                        continue
                    relu_col(b_, tt)
                    cnt += 1
                    if cnt % 6 == 0:
                        maybe_U(1)
                softmax_awe_b(b_)
                maybe_U(1)
            maybe_U(len(uq))

            # ---------- gate: g01T = [g0, g1]/SCL ----------
            def fc_feat(kc):
                return hid_cols(kc) if kc < 8 else aweT[:, kc - 8, :]

            glT = ps_g.tile([128, 1], dt.float32, tag="small", bufs=2,
                            name="glt")
            for kc in range(12):
                nc.tensor.matmul(glT[:], fc_feat(kc),
                                 wdiff_sb[:, kc:kc + 1],
                                 start=(kc == 0), stop=False)
            nc.tensor.matmul(glT[:], ones1b_sb[:], bdiff_sb[:],
                             start=False, stop=True)
            gex = work.tile([128, 1], dt.float32, tag="gex", bufs=1,
                            name="gex")
            nc.scalar.activation(gex[:], glT[:], AF.Exp, scale=-1.0)
            # g0/SCL = 1/((1+gex)*SCL); g1/SCL = 1/SCL - g0/SCL
            nc.vector.tensor_scalar(out=gex[:], in0=gex[:], scalar1=1.0,
                                    scalar2=SCL, op0=ALU.add, op1=ALU.mult)
            nc.vector.reciprocal(g01T[:, 0:1], gex[:])
            nc.vector.tensor_scalar(out=g01T[:, 1:2], in0=g01T[:, 0:1],
                                    scalar1=-1.0, scalar2=DQ,
                                    op0=ALU.mult, op1=ALU.add)

            # ---------- fc awe-part + gate combine + writeback (bf16) ------
            for vb in range(NVB):
                v0 = vb * 512
                vn = min(512, V - v0)
                wt = wtA[vb]
                pt = ps_mm.tile([128, 512], dt.float32, tag="pmm")
                for kc in range(4):
                    nc.tensor.matmul(pt[:, 0:vn], aweT[:, kc, :],
                                     wt[:, kc, 0:vn],
                                     start=(kc == 0), stop=False)
                nc.tensor.matmul(pt[:, 0:vn], ones1b_sb[:],
                                 bfcrow_sb[0:1, v0:v0 + vn],
                                 start=False, stop=True)
                ost = outp.tile([128, 512], dt.bfloat16, tag="ost")
                nc.scalar.activation(ost[:, 0:vn], U_sb[vb][:, 0:vn],
                                     AF.Identity, scale=g01T[:, 0:1])
                nc.vector.scalar_tensor_tensor(
                    out=ost[:, 0:vn], in0=pt[:, 0:vn],
                    scalar=g01T[:, 1:2], in1=ost[:, 0:vn],
                    op0=ALU.mult, op1=ALU.add)
                dst = bass.AP(tensor=out_t[:].tensor, offset=v0,
                              ap=[[V, 128], [1, vn]])
                # alternate ACT/Pool DMA queues: off the head-blocked sync
                # queue, and neither queue eats all the serialization
                if vb % 2 == 0:
                    nc.gpsimd.dma_start(out=dst, in_=ost[:, 0:vn])
                else:
                    nc.scalar.dma_start(out=dst, in_=ost[:, 0:vn])

    nc.compile()
    return nc


def _host_prep(inputs):
    f32 = np.float32

    def bf(x):
        return np.ascontiguousarray(np.asarray(x, f32).astype(BF))

    def q8(x):
        # fixed power-of-2 scale + clip; RNE via astype
        y = np.clip(np.asarray(x, f32) * SCL, -E3MAX, E3MAX)
        return np.ascontiguousarray(y.astype(E3))

    enc = np.asarray(inputs["encoder_out"], f32)
    enc_p = enc.reshape(B, E, PP)
    feats_all = enc.mean(axis=2)                  # (B, E, W=T)

    # permute gate blocks (i,f,g,o) -> (i,f,o,g) so one sigmoid spans i,f,o
    gp = np.r_[0:2 * D, 3 * D:4 * D, 2 * D:3 * D]

    common = {}
    common["wih1f"] = q8(np.asarray(inputs["Wih1"], f32).T[:, gp])
    common["wih1r"] = q8(np.asarray(inputs["Wih1r"], f32).T[:, gp])
    common["whh1f"] = q8(np.asarray(inputs["Whh1"], f32).T[:, gp])
    common["whh1r"] = q8(np.asarray(inputs["Whh1r"], f32).T[:, gp])
    common["wih2f"] = q8(np.asarray(inputs["Wih2"], f32).T[:, gp])
    common["wih2r"] = q8(np.asarray(inputs["Wih2r"], f32).T[:, gp])
    common["whh2f"] = q8(np.asarray(inputs["Whh2"], f32).T[:, gp])
    common["whh2r"] = q8(np.asarray(inputs["Whh2r"], f32).T[:, gp])
    common["b1f"] = bf(np.asarray(inputs["bih1"] + inputs["bhh1"],
                                  f32)[gp] * SCL)
    common["b1r"] = bf(np.asarray(inputs["bih1r"] + inputs["bhh1r"],
                                  f32)[gp] * SCL)
    common["b2f"] = bf(np.asarray(inputs["bih2"] + inputs["bhh2"],
                                  f32)[gp] * SCL)
    common["b2r"] = bf(np.asarray(inputs["bih2r"] + inputs["bhh2r"],
                                  f32)[gp] * SCL)
    common["wencT"] = q8(np.asarray(inputs["Wenc"], f32).T)
    common["wdecT"] = q8(np.asarray(inputs["Wdec"], f32).T)
    common["beab"] = bf(np.asarray(inputs["benc"] + inputs["bdec"],
                                   f32) * SCL)
    common["wfullb"] = bf(np.asarray(inputs["Wfull"], f32)[0])
    wg = np.asarray(inputs["Wg"], f32)
    common["wdiffT"] = bf(wg[0] - wg[1])
    bd = float(np.asarray(inputs["bg"], f32)[0]
               - np.asarray(inputs["bg"], f32)[1])
    common["bdiffb"] = bf(np.array([[bd]], f32))
    common["eye128"] = bf(np.eye(128, dtype=f32))
    common["wfcT"] = q8(np.asarray(inputs["Wfc"], f32).T)
    bfc = np.zeros(VCH * 128, f32)
    bfc[:V] = np.asarray(inputs["bfc"], f32)
    common["bfcp"] = bf(bfc * SCL)

    in_maps = []
    for c in range(NCORES):
        m = dict(common)
        sl = enc_p[c * NB:(c + 1) * NB]
        m["enc_ep"] = bf(sl)
        m["enc_pe"] = bf(np.ascontiguousarray(sl.transpose(0, 2, 1)))
        m["featsb"] = bf(np.ascontiguousarray(
            feats_all[c * NB:(c + 1) * NB].transpose(1, 0, 2)))
        in_maps.append(m)
    return in_maps


def _get_program():
    if "nc" not in _prog_cache:
        _prog_cache["nc"] = _build_program()
    return _prog_cache["nc"]


def kernel(**inputs):
    from concourse.bass_utils import run_bass_kernel_spmd

    nc = _get_program()
    in_maps = _host_prep(inputs)
    res = run_bass_kernel_spmd(nc, in_maps, list(range(NCORES)))
    # per-core result is [b, t, v] bf16; assemble to (T, B, V) f32
    out = np.concatenate(
        [np.asarray(res.results[c]["out"], np.float32).transpose(1, 0, 2)
         for c in range(NCORES)],
        axis=1)
    return np.ascontiguousarray(out, np.float32)


# revision 31
# speedup vs baseline: 1.1676x; 1.0074x over previous
"""Trainium2 Bass kernel for DecoderWithAttention (bidirectional 2-layer LSTM +
additive attention + gated fc), data-parallel over batch across 8 NeuronCores.

Shapes (hardcoded): encoder_out (64, 512, 16, 16), T=16, D=A=512, V=5000.
Per core: 8 batches, full network, weights replicated (no collectives under
this axon terminal, so each core is fully independent).

Major layout/optimization decisions (v2):
  - All large weights (Wih*/Whh*/Wenc/Wdec/Wfc) are stored+streamed as
    float8 e3m4, host-scaled by a fixed power-of-2 (SCL=32, clipped) so the
    fp8 mantissa window is centered; dequant is folded into ACT scale params
    (gates: sigmoid/tanh(psum/SCL); softmax: exp(sc/SCL); fc: 1/SCL folded
    into the gate coefficients g0/g1).  Halves the ~43MB/core DMA to ~20MB.
  - feats (AdaptiveAvgPool over H) is computed host-side (input prep),
    removing the on-chip reduce and letting L1 start ~10us earlier.
  - LSTM step: psum groups (g | ifo) -> 3 ACT ops/step (tanh_g, sig_ifo,
    tanh_c); both directions fused in each op; h written by DVE (fwd) and
    Pool (rev).  Xp psum->SBUF copies ride on Pool/DVE.
  - Weights pinned in SBUF (no rotation pool), so every weight DMA issues
    back-to-back with no WAR stalls; wfc hidden/awe parts stream through
    small fp8 rotation pools ordered so the sync queue never head-blocks.
  - Attention: att1w=Wenc^T enc precomputed (scheduler lifts it into L1-rec
    PE idle); att2 per-timestep inside the L2 loop; relu/score columns for
    b0/b1 interleaved into the L2 loop (DVE/ACT/Pool), the rest deferred and
    engine-balanced; softmax+awe pipelined per-batch behind the relu columns.
  - fc factorized around the gate (g0*(hid Wh+bfc) + g1*(awe Wa+bfc)); the
    hidden part streams right after the L2 recurrence; gate sigmoid folds
    1/SCL; output written bf16 (host upcasts to f32).
"""

import numpy as np
import ml_dtypes

BF = ml_dtypes.bfloat16
E3 = ml_dtypes.float8_e3m4
B, E, HH, WW = 64, 512, 16, 16
T = WW          # 16 timesteps
PP = HH * WW    # 256 attention positions
D = 512
A = 512
V = 5000
G = 4 * D
NB = 8          # batches per core
NCORES = 8
F = 2 * D + E   # 1536
VCH = (V + 127) // 128  # 40
NVB = 10        # fc v-blocks of 512

SCL = 32.0      # fixed power-of-2 fp8 scale for all quantized weights
DQ = 1.0 / SCL  # dequant factor (exact)
E3MAX = 15.5

_prog_cache = {}


def _build_program():
    import concourse.bass as bass
    import concourse.bacc as bacc
    import concourse.mybir as mybir
    import concourse.tile as tile

    dt = mybir.dt
    AF = mybir.ActivationFunctionType
    ALU = mybir.AluOpType

    nc = bacc.Bacc("TRN2", target_bir_lowering=False, debug=False,
                   num_devices=NCORES, dynamic_dma_scratch_size=2048)

    def din(name, shape, d=dt.bfloat16):
        return nc.dram_tensor(name, shape, d, kind="ExternalInput")

    f8 = dt.float8e3
    featsb = din("featsb", [E, NB, T])            # host mean over H, bf16
    enc_ep = din("enc_ep", [NB, E, PP])           # [b, e, p]
    enc_pe = din("enc_pe", [NB, PP, E])           # [b, p, e]
    wih1 = {0: din("wih1f", [E, G], f8), 1: din("wih1r", [E, G], f8)}
    whh1 = {0: din("whh1f", [D, G], f8), 1: din("whh1r", [D, G], f8)}
    wih2 = {0: din("wih2f", [2 * D, G], f8), 1: din("wih2r", [2 * D, G], f8)}
    whh2 = {0: din("whh2f", [D, G], f8), 1: din("whh2r", [D, G], f8)}
    b1 = {0: din("b1f", [G]), 1: din("b1r", [G])}   # (bih+bhh)*SCL bf16
    b2 = {0: din("b2f", [G]), 1: din("b2r", [G])}
    wencT = din("wencT", [E, A], f8)
    wdecT = din("wdecT", [2 * D, A], f8)
    beab = din("beab", [A])                       # (benc+bdec)*SCL, bf16
    wfullb = din("wfullb", [A])                   # Wfull[0] bf16 (signed)
    wdiffT = din("wdiffT", [F])                   # Wg[0]-Wg[1] bf16
    bdiffb = din("bdiffb", [1, 1])                # bg0-bg1 bf16
    eye128 = din("eye128", [128, 128])
    wfcT = din("wfcT", [F, V], f8)
    bfcp = din("bfcp", [VCH * 128])               # bfc*SCL, bf16 padded
    out_t = nc.dram_tensor("out", [NB, T, V], dt.bfloat16,
                           kind="ExternalOutput")

    with tile.TileContext(nc) as tc:
        with (
            tc.tile_pool(name="const", bufs=1) as const,
            tc.tile_pool(name="work", bufs=4) as work,
            tc.tile_pool(name="rwp", bufs=10) as rwp,
            tc.tile_pool(name="wfcp", bufs=2) as wfcp,
            tc.tile_pool(name="outp", bufs=3) as outp,
            tc.tile_pool(name="ps_g", bufs=1, space="PSUM") as ps_g,
            tc.tile_pool(name="ps_mm", bufs=3, space="PSUM") as ps_mm,
            tc.tile_pool(name="ps_sc", bufs=1, space="PSUM") as ps_sc,
        ):
            dma = nc.sync.dma_start

            # ---------------- DMA section (sync-queue program order) -------
            # tiny tiles the early phases depend on go first
            feats = const.tile([128, 4, NB, T], dt.bfloat16)   # (ech, b, t)
            dma(out=feats[:],
                in_=featsb[:].rearrange("(ec ep) b t -> ep ec b t", ep=128))
            eye_sb = const.tile([128, 128], dt.bfloat16)
            dma(out=eye_sb[:], in_=eye128[:])
            b1row, b2row = {}, {}
            for d_ in (0, 1):
                b1row[d_] = const.tile([1, G], dt.bfloat16, tag=f"b1r_{d_}",
                                       bufs=1, name=f"b1row{d_}")
                dma(out=b1row[d_][:], in_=b1[d_][:])

            # pinned fp8 LSTM weights.  L1 weights are split into two
            # half-G tiles each so the first projection / recurrence matmuls
            # only wait on the first half-MB DMA.
            def wload(dram, kchunks, nm, splits=1):
                gs = G // splits
                ts = []
                for i in range(splits):
                    t_ = const.tile([128, kchunks, gs], f8, name=f"{nm}_{i}")
                    dma(out=t_[:],
                        in_=dram[:, i * gs:(i + 1) * gs]
                        .rearrange("(kc kp) g -> kp kc g", kp=128))
                    ts.append(t_)
                return ts

            def wsl(wsb, d_, kc, mch):
                # weight slice [128, 128] for gate chunk mch
                ts = wsb[d_]
                n = len(ts)
                per = 16 // n
                t_ = ts[mch // per]
                j = mch % per
                return t_[:, kc, j * 128:(j + 1) * 128]

            wih1_sb = {d_: wload(wih1[d_], 4, f"wih1_{d_}", 2)
                       for d_ in (0, 1)}
            whh1_sb = {d_: wload(whh1[d_], 4, f"whh1_{d_}", 2)
                       for d_ in (0, 1)}
            wencT_sb = const.tile([128, 4, A], f8)            # (ech, a)
            dma(out=wencT_sb[:],
                in_=wencT[:].rearrange("(ec ep) a -> ep ec a", ep=128))
            enc_ep_sb = const.tile([128, NB, 4, PP], dt.bfloat16,
                                   tag="encbuf", bufs=1, name="enc_ep")
            for bh in (0, 1):
                dma(out=enc_ep_sb[:, 4 * bh:4 * bh + 4, :, :],
                    in_=enc_ep[4 * bh:4 * bh + 4]
                    .rearrange("b (ec ep) p -> ep b ec p", ep=128))
            for d_ in (0, 1):
                b2row[d_] = const.tile([1, G], dt.bfloat16, tag=f"b2r_{d_}",
                                       name=f"b2row{d_}")
                dma(out=b2row[d_][:], in_=b2[d_][:])
            wih2_sb = {d_: wload(wih2[d_], 8, f"wih2_{d_}") for d_ in (0, 1)}
            whh2_sb = {d_: wload(whh2[d_], 4, f"whh2_{d_}") for d_ in (0, 1)}
            # prime the tanh/sigmoid ACT table during the DMA head so the
            # first gate activation doesn't eat a 1.3us table load
            prime = work.tile([1, 1], dt.float32, tag="dmy", bufs=1,
                              name="prime")
            nc.scalar.activation(prime[:], eye_sb[0:1, 0:1], AF.Tanh)
            wdecT_sb = const.tile([128, 8, A], f8)            # (kch, a)
            dma(out=wdecT_sb[:],
                in_=wdecT[:].rearrange("(kc kp) a -> kp kc a", kp=128))
            wrow_sb = const.tile([128, 4], dt.bfloat16)       # Wfull (signed)
            dma(out=wrow_sb[:], in_=wfullb[:].rearrange("(c p) -> p c", p=128))
            bea_row = const.tile([1, A], dt.bfloat16)
            dma(out=bea_row[:], in_=beab[:])
            wdiff_sb = const.tile([128, 12], dt.bfloat16)
            dma(out=wdiff_sb[:], in_=wdiffT[:].rearrange("(c p) -> p c", p=128))
            bdiff_sb = const.tile([1, 1], dt.bfloat16)
            dma(out=bdiff_sb[:], in_=bdiffb[:])
            bfcrow_sb = const.tile([1, VCH * 128], dt.bfloat16)
            dma(out=bfcrow_sb[:], in_=bfcp[:])
            # enc_pe shares the enc_ep slot (same 16KB); its DMA waits until
            # the att1w matmuls (the only enc_ep readers) are done
            enc_pe_sb = const.tile([128, NB, 2, E], dt.bfloat16,
                                   tag="encbuf", name="enc_pe")
            dma(out=enc_pe_sb[:],
                in_=enc_pe[:].rearrange("b (pc pp) e -> pp b pc e", pp=128))

            # fc weight streams (fp8). wfcU rotation: first 6 issue during
            # the recurrences, the rest pace with the U matmuls.  wfcA's
            # first 4 issue early; the rest pace with the A matmuls.  Order
            # keeps sync-queue head-blocking monotone with need times.
            wtU, wtA = [], []

            def load_U(vb):
                wt = wfcp.tile([128, 8, 512], f8, tag="wfcU", bufs=5,
                               name="wtU")
                v0 = vb * 512
                vn = min(512, V - v0)
                dma(out=wt[:, :, 0:vn],
                    in_=wfcT[0:2 * D, v0:v0 + vn]
                    .rearrange("(kc kp) v -> kp kc v", kp=128))
                wtU.append(wt)

            def load_A(vb):
                wt = wfcp.tile([128, 4, 512], f8, tag="wfcA", bufs=2,
                               name="wtA")
                v0 = vb * 512
                vn = min(512, V - v0)
                dma(out=wt[:, :, 0:vn],
                    in_=wfcT[2 * D:F, v0:v0 + vn]
                    .rearrange("(kc kp) v -> kp kc v", kp=128))
                wtA.append(wt)

            for vb in range(5):
                load_U(vb)
            for vb in range(2):
                load_A(vb)

            def load_A_slot(vb, tag, nm):
                wt = const.tile([128, 4, 512], f8, tag=tag, name=nm)
                v0 = vb * 512
                vn = min(512, V - v0)
                dma(out=wt[:, :, 0:vn],
                    in_=wfcT[2 * D:F, v0:v0 + vn]
                    .rearrange("(kc kp) v -> kp kc v", kp=128))
                wtA.append(wt)

            # ---------------- persistent SBUF state ----------------
            Xp1 = {d_: const.tile([128, 16, NB, T], dt.bfloat16,
                                  tag=f"xp1_{d_}", bufs=1, name=f"Xp1_{d_}")
                   for d_ in (0, 1)}                          # (gch, b, t)
            Xp2 = {d_: const.tile([128, 16, T, NB], dt.bfloat16,
                                  tag=f"xp2_{d_}", bufs=1, name=f"Xp2_{d_}")
                   for d_ in (0, 1)}                          # (gch, t, b)
            H1 = {d_: const.tile([128, 4, T, NB], dt.bfloat16,
                                 tag=f"h1_{d_}", name=f"H1_{d_}")
                  for d_ in (0, 1)}                           # (dch, t, b)
            H2 = {d_: const.tile([128, 4, T, NB], dt.bfloat16,
                                 tag=f"h2_{d_}", name=f"H2_{d_}")
                  for d_ in (0, 1)}
            att1w = const.tile([128, NB, 4, PP], dt.bfloat16)  # (b, ach, p)
            att2pb = const.tile([128, 4, 128], dt.float32)     # (ach, (b,t))
            att2pb_r = att2pb[:].rearrange("p a (b t) -> p a t b", t=T)
            E_sb = const.tile([128, 2, 128], dt.bfloat16)      # exp(sc/SCL)
            alphaT = const.tile([128, 2, 128], dt.bfloat16)    # (pch, (b,t))
            aweT = const.tile([128, 4, 128], dt.bfloat16)      # (ech, (b,t))
            recip_sb = const.tile([1, 128], dt.float32)
            U_sb = [const.tile([128, 512], dt.bfloat16, tag=f"u{vb}",
                               name=f"U{vb}") for vb in range(NVB)]
            g01T = const.tile([128, 2], dt.float32)
            ones1_sb = const.tile([1, 128], dt.float32)
            nc.vector.memset(ones1_sb[:], 1.0)
            ones1b_sb = const.tile([1, 128], dt.bfloat16)
            nc.vector.memset(ones1b_sb[:], 1.0)
            ones_sb = const.tile([128, 1], dt.bfloat16)
            nc.vector.memset(ones_sb[:], 1.0)

            Hw = const.tile([128, 8, 128], dt.bfloat16)  # (kch, (b,t))

            def hid_cols(kc):
                return Hw[:, kc, :]

            # ---------- layer-1 input projections (all t, N=128) ----------
            for d_ in (0, 1):
                for mp in range(8):
                    pt = ps_mm.tile([128, 512], dt.float32, tag="pmm")
                    for half in (0, 1):
                        mch = 2 * mp + half
                        sl = pt[:, half * 128:(half + 1) * 128]
                        for kc in range(4):
                            nc.tensor.matmul(
                                sl,
                                wsl(wih1_sb, d_, kc, mch),
                                feats[:, kc, :, :], start=(kc == 0),
                                stop=False)
                        nc.tensor.matmul(
                            sl, b1row[d_][0:1, mch * 128:(mch + 1) * 128],
                            ones1b_sb[:], start=False, stop=True)
                    # gpsimd cannot read PSUM on hw; alternate DVE/ACT
                    if mp % 2 == 0:
                        nc.vector.tensor_copy(
                            Xp1[d_][:, 2 * mp:2 * mp + 2, :, :]
                            .rearrange("p m b w -> p (m b w)"), pt[:, 0:256])
                    else:
                        nc.scalar.copy(
                            Xp1[d_][:, 2 * mp:2 * mp + 2, :, :]
                            .rearrange("p m b w -> p (m b w)"), pt[:, 0:256])

            # ---------- LSTM fused step pair ----------
            # Gate blocks host-permuted to (i, f, o, g):
            # ch 0-3=i, 4-7=f, 8-11=o, 12-15=g.
            # psum groups: g (tanh feeds ig first) | ifo (single sigmoid).
            def step_pair(wsb, xps, Hs, c_tile, s):
                pg_g = ps_g.tile([128, 2, 4, NB], dt.float32, tag="pgg",
                                 bufs=1, name="pgg")
                pg_ifo = ps_g.tile([128, 2, 12, NB], dt.float32, tag="pgifo",
                                   bufs=1, name="pgifo")
                for pt_, mlo, nch in ((pg_g, 12, 4), (pg_ifo, 0, 12)):
                    for d_ in (0, 1):
                        t_log = s if d_ == 0 else T - 1 - s
                        t_prev = t_log - 1 if d_ == 0 else t_log + 1
                        h_prev = None if s == 0 else Hs[d_][:, :, t_prev, :]
                        for j in range(nch):
                            mch = mlo + j
                            if h_prev is not None:
                                for kc in range(4):
                                    nc.tensor.matmul(
                                        pt_[:, d_, j, :],
                                        wsl(wsb, d_, kc, mch),
                                        h_prev[:, kc, :],
                                        start=(kc == 0), stop=False)
                            nc.tensor.matmul(
                                pt_[:, d_, j, :], eye_sb[:],
                                xps[d_][:, mch, :],
                                start=(s == 0), stop=True)
                ga_g = work.tile([128, 2, 4, NB], dt.float32, tag="gag",
                                 bufs=2, name="gag")
                ga_ifo = work.tile([128, 2, 12, NB], dt.float32, tag="gaifo",
                                   bufs=3, name="gaifo")
                nc.scalar.activation(ga_g[:], pg_g[:], AF.Tanh, scale=DQ)
                nc.scalar.activation(ga_ifo[:], pg_ifo[:], AF.Sigmoid,
                                     scale=DQ)
                ig = work.tile([128, 2, 4, NB], dt.float32, tag="ig",
                               bufs=2, name="ig")
                nc.vector.tensor_tensor(out=ig[:], in0=ga_ifo[:, :, 0:4, :],
                                        in1=ga_g[:], op=ALU.mult)
                if s == 0:
                    nc.vector.tensor_copy(c_tile[:], ig[:])
                else:
                    nc.vector.tensor_tensor(out=c_tile[:], in0=c_tile[:],
                                            in1=ga_ifo[:, :, 4:8, :],
                                            op=ALU.mult)
                    nc.vector.tensor_tensor(out=c_tile[:], in0=c_tile[:],
                                            in1=ig[:], op=ALU.add)
                th = work.tile([128, 2, 4, NB], dt.float32, tag="th",
                               bufs=2, name="th")
                nc.scalar.activation(th[:], c_tile[:], AF.Tanh)
                for d_ in (0, 1):
                    t_log = s if d_ == 0 else T - 1 - s
                    eng = nc.vector if d_ == 0 else nc.gpsimd
                    eng.tensor_tensor(out=Hs[d_][:, :, t_log, :],
                                      in0=th[:, d_, :, :],
                                      in1=ga_ifo[:, d_, 8:12, :],
                                      op=ALU.mult)

            # ---------- layer-1 recurrence ----------
            c1 = work.tile([128, 2, 4, NB], dt.float32, tag="c1", bufs=1,
                           name="c1")
            for s in range(T):
                step_pair(whh1_sb, {
                    0: Xp1[0][:, :, :, s],
                    1: Xp1[1][:, :, :, T - 1 - s]}, H1, c1, s)

            # awe-part weights vb2-3 ride the retired Xp1 slots (their
            # L1-recurrence readers are all emitted above)
            load_A_slot(2, "xp1_0", "wtAx0")
            load_A_slot(3, "xp1_1", "wtAx1")

            # ---------- att1w = satt*Wenc^T enc  (fills L1-rec PE idle) ----
            for ac in range(4):
                for bblk in range(4):
                    pt = ps_mm.tile([128, 512], dt.float32, tag="pmm",
                                    name="pta1")
                    for bh in (0, 1):
                        b_ = 2 * bblk + bh
                        for ec in range(4):
                            nc.tensor.matmul(
                                pt[:, bh * 256:(bh + 1) * 256],
                                wencT_sb[:, ec, ac * 128:(ac + 1) * 128],
                                enc_ep_sb[:, b_, ec, :],
                                start=(ec == 0), stop=(ec == 3))
                    if (ac + bblk) % 2 == 0:
                        nc.vector.tensor_copy(
                            att1w[:, 2 * bblk:2 * bblk + 2, ac, :], pt[:])
                    else:
                        nc.scalar.copy(
                            att1w[:, 2 * bblk:2 * bblk + 2, ac, :], pt[:])

            # ---------- layer-2 input projections ----------
            for d_ in (0, 1):
                for mp in range(8):
                    pt = ps_mm.tile([128, 512], dt.float32, tag="pmm")
                    for half in (0, 1):
                        mch = 2 * mp + half
                        sl = pt[:, half * 128:(half + 1) * 128]
                        for kc in range(8):
                            rhs = (H1[0] if kc < 4 else H1[1])[:, kc % 4, :, :]
                            nc.tensor.matmul(
                                sl,
                                wsl(wih2_sb, d_, kc, mch),
                                rhs, start=(kc == 0), stop=False)
                        nc.tensor.matmul(
                            sl, b2row[d_][0:1, mch * 128:(mch + 1) * 128],
                            ones1b_sb[:], start=False, stop=True)
                    if mp % 2 == 0:
                        nc.vector.tensor_copy(
                            Xp2[d_][:, 2 * mp:2 * mp + 2, :, :]
                            .rearrange("p m t b -> p (m t b)"), pt[:, 0:256])
                    else:
                        nc.scalar.copy(
                            Xp2[d_][:, 2 * mp:2 * mp + 2, :, :]
                            .rearrange("p m t b -> p (m t b)"), pt[:, 0:256])

            # ---------- attention helpers ----------
            sc_t = ps_sc.tile([128, 2, 128], dt.float32, tag="sc",
                              name="scps")

            def emit_att2_pair(s):
                # both completed timesteps (15-s, s) in one 16-col matmul set
                lo, st = T - 1 - s, 2 * s - (T - 1)
                tsl = slice(lo, s + 1, st)
                pa2 = ps_g.tile([128, 4, 2, NB], dt.float32, tag="small",
                                bufs=2, name="pa2")
                for ac in range(4):
                    for kc in range(8):
                        nc.tensor.matmul(
                            pa2[:, ac, :, :],
                            wdecT_sb[:, kc, ac * 128:(ac + 1) * 128],
                            (H2[0] if kc < 4 else H2[1])[:, kc % 4, tsl, :],
                            start=(kc == 0), stop=False)
                    nc.tensor.matmul(
                        pa2[:, ac, :, :],
                        bea_row[0:1, ac * 128:(ac + 1) * 128],
                        ones1b_sb[0:1, 0:2 * NB].rearrange(
                            "o (u b) -> o u b", u=2),
                        start=False, stop=True)
                nc.vector.tensor_copy(att2pb_r[:, :, tsl, :], pa2[:])

            def emit_rw_col(b_, tt, engs):
                if isinstance(engs, int):
                    engs = (engs,) * 4
                col = b_ * T + tt
                rws = []
                for ac in range(4):
                    rw = rwp.tile([128, PP], dt.bfloat16, tag="rw")
                    eng = engs[ac]
                    if eng == 0:
                        nc.vector.tensor_scalar(
                            out=rw[:], in0=att1w[:, b_, ac, :],
                            scalar1=att2pb[:, ac, col:col + 1],
                            scalar2=0.0, op0=ALU.add, op1=ALU.max)
                    elif eng == 1:
                        nc.scalar.activation(
                            rw[:], att1w[:, b_, ac, :], AF.Relu,
                            bias=att2pb[:, ac, col:col + 1])
                    else:
                        nc.gpsimd.tensor_scalar(
                            out=rw[:], in0=att1w[:, b_, ac, :],
                            scalar1=att2pb[:, ac, col:col + 1],
                            scalar2=0.0, op0=ALU.add, op1=ALU.max)
                    rws.append(rw)
                # ph-major so the two accumulation groups in sc_t's single
                # psum zero-region never overlap (one must close before the
                # other starts)
                for ph in range(2):
                    for ac in range(4):
                        nc.tensor.matmul(
                            sc_t[:, ph, col:col + 1],
                            rws[ac][:, ph * 128:(ph + 1) * 128],
                            wrow_sb[:, ac:ac + 1],
                            start=(ac == 0), stop=(ac == 3))

            done_cols = set()

            # ---------- layer-2 recurrence (att2/relu interleaved) ----------
            c2 = work.tile([128, 2, 4, NB], dt.float32, tag="c2", bufs=1,
                           name="c2")
            for s in range(T):
                step_pair(whh2_sb, {
                    0: Xp2[0][:, :, s, :],
                    1: Xp2[1][:, :, T - 1 - s, :]}, H2, c2, s)
                if s >= 8:
                    emit_att2_pair(s)
                    # 3 columns/step interleaved, ops split across engines
                    # so no single engine's step budget is blown
                    for b_, tt, engs in ((0, s, (0, 0, 0, 0)),
                                         (0, T - 1 - s, (0, 0, 0, 2)),
                                         (1, s, (1, 1, 2, 2))):
                        emit_rw_col(b_, tt, engs)
                        done_cols.add((b_, tt))

            # remaining fc weight streams: wfcU 5-9 pace with the U
            # matmuls; wfcA 4-5 ride the retired Xp2 slots; 6-9 rotate the
            # wfcA pool behind the A matmuls
            for vb in range(5, NVB):
                load_U(vb)
            load_A_slot(4, "xp2_0", "wtAx2")
            load_A_slot(5, "xp2_1", "wtAx3")
            for vb in range(6, NVB):
                load_A(vb)

            # hidden in (b,t)-column order for fc/gate stationary operands
            # (a stationary AP must have a single free dim, so the permuted
            # view must be materialized); SBUF->SBUF, Pool/DVE/ACT mix
            for kc in range(8):
                src_ = (H2[0] if kc < 4 else H2[1])[:, kc % 4, :, :] \
                    .rearrange("p t b -> p b t")
                if kc % 4 == 3:
                    nc.vector.tensor_copy(Hw[:, kc, :], src_)
                elif kc % 4 == 2:
                    nc.scalar.copy(Hw[:, kc, :], src_)
                else:
                    nc.gpsimd.tensor_copy(Hw[:, kc, :], src_)

            # switch the ACT table to exp_and_others (relu/tanh/copy stay
            # available in it; sigmoid is no longer needed)
            dmy = work.tile([1, 1], dt.float32, tag="dmy", bufs=1, name="dmy")
            nc.scalar.activation(dmy[:], ones1_sb[0:1, 0:1], AF.Exp)

            # ---------- tail: U-phase || deferred relu || per-b softmax ----
            # U matmuls, the deferred relu columns, and the per-batch
            # softmax/awe are emission-interleaved so PE (in-order) streams
            # the fc hidden part WHILE the elementwise engines chew relu.
            accD = accA = accP = 0.0   # engine-balance accumulators (ns)

            def emit_U(vb):
                nonlocal accA
                v0 = vb * 512
                vn = min(512, V - v0)
                wt = wtU[vb]
                pt = ps_mm.tile([128, 512], dt.float32, tag="pmm")
                for kc in range(8):
                    nc.tensor.matmul(pt[:, 0:vn], hid_cols(kc),
                                     wt[:, kc, 0:vn],
                                     start=(kc == 0), stop=False)
                nc.tensor.matmul(pt[:, 0:vn], ones1b_sb[:],
                                 bfcrow_sb[0:1, v0:v0 + vn],
                                 start=False, stop=True)
                nc.scalar.copy(U_sb[vb][:, 0:vn], pt[:, 0:vn])
                accA += 612.0

            def relu_col(b_, tt):
                nonlocal accD, accA, accP
                engs = []
                for _ in range(4):
                    costs = (accD + 127.0, accA + 398.0, accP + 544.0)
                    eng = int(np.argmin(costs))
                    if eng == 0:
                        accD = costs[0]
                    elif eng == 1:
                        accA = costs[1]
                    else:
                        accP = costs[2]
                    engs.append(eng)
                emit_rw_col(b_, tt, tuple(engs))

            def softmax_awe_b(b_):
                cs = slice(b_ * T, (b_ + 1) * T)
                nc.scalar.activation(E_sb[:, :, cs], sc_t[:, :, cs], AF.Exp,
                                     scale=DQ)
                sums = ps_g.tile([1, T], dt.float32, tag="small", bufs=2,
                                 name="sums")
                for ph in range(2):
                    nc.tensor.matmul(sums[:], ones_sb[:], E_sb[:, ph, cs],
                                     start=(ph == 0), stop=(ph == 1))
                nc.vector.reciprocal(recip_sb[0:1, cs], sums[:])
                bc = ps_g.tile([128, T], dt.float32, tag="small", bufs=2,
                               name="bc")
                nc.tensor.matmul(bc[:], ones1_sb[:], recip_sb[0:1, cs],
                                 start=True, stop=True)
                for ph in range(2):
                    nc.vector.tensor_tensor(out=alphaT[:, ph, cs],
                                            in0=E_sb[:, ph, cs],
                                            in1=bc[:], op=ALU.mult)
                pa = ps_g.tile([128, 4, T], dt.float32, tag="small", bufs=2,
                               name="pab")
                for ec in range(4):
                    for pc in range(2):
                        nc.tensor.matmul(
                            pa[:, ec, :],
                            enc_pe_sb[:, b_, pc, ec * 128:(ec + 1) * 128],
                            alphaT[:, pc, cs],
                            start=(pc == 0), stop=(pc == 1))
                nc.vector.tensor_copy(aweT[:, :, cs], pa[:])
                nonlocal accD, accA, accP
                accA += 170.0
                accD += 330.0
                accP += 250.0

            uq = list(range(NVB))

            def maybe_U(n=1):
                for _ in range(n):
                    if uq:
                        emit_U(uq.pop(0))

            softmax_awe_b(0)
            maybe_U(1)
            for b_ in range(1, NB):
                cnt = 0
                for tt in range(T):
                    if (b_, tt) in done_cols:
                        continue
                    relu_col(b_, tt)
                    cnt += 1
                    if cnt % 6 == 0:
                        maybe_U(1)
                softmax_awe_b(b_)
                maybe_U(1)
            maybe_U(len(uq))

            # ---------- gate: g01T = [g0, g1]/SCL ----------
            def fc_feat(kc):
                return hid_cols(kc) if kc < 8 else aweT[:, kc - 8, :]

            glT = ps_g.tile([128, 1], dt.float32, tag="small", bufs=2,
                            name="glt")
            for kc in range(12):
                nc.tensor.matmul(glT[:], fc_feat(kc),
                                 wdiff_sb[:, kc:kc + 1],
                                 start=(kc == 0), stop=False)
            nc.tensor.matmul(glT[:], ones1b_sb[:], bdiff_sb[:],
                             start=False, stop=True)
            gex = work.tile([128, 1], dt.float32, tag="gex", bufs=1,
                            name="gex")
            nc.scalar.activation(gex[:], glT[:], AF.Exp, scale=-1.0)
            # g0/SCL = 1/((1+gex)*SCL); g1/SCL = 1/SCL - g0/SCL
            nc.vector.tensor_scalar(out=gex[:], in0=gex[:], scalar1=1.0,
                                    scalar2=SCL, op0=ALU.add, op1=ALU.mult)
            nc.vector.reciprocal(g01T[:, 0:1], gex[:])
            nc.vector.tensor_scalar(out=g01T[:, 1:2], in0=g01T[:, 0:1],
                                    scalar1=-1.0, scalar2=DQ,
                                    op0=ALU.mult, op1=ALU.add)

            # ---------- fc awe-part + gate combine + writeback (bf16) ------
            for vb in range(NVB):
                v0 = vb * 512
                vn = min(512, V - v0)
                wt = wtA[vb]
                pt = ps_mm.tile([128, 512], dt.float32, tag="pmm")
                for kc in range(4):
                    nc.tensor.matmul(pt[:, 0:vn], aweT[:, kc, :],
                                     wt[:, kc, 0:vn],
                                     start=(kc == 0), stop=False)
                nc.tensor.matmul(pt[:, 0:vn], ones1b_sb[:],
                                 bfcrow_sb[0:1, v0:v0 + vn],
                                 start=False, stop=True)
                ost = outp.tile([128, 512], dt.bfloat16, tag="ost")
                nc.scalar.activation(ost[:, 0:vn], U_sb[vb][:, 0:vn],
                                     AF.Identity, scale=g01T[:, 0:1])
                nc.vector.scalar_tensor_tensor(
                    out=ost[:, 0:vn], in0=pt[:, 0:vn],
                    scalar=g01T[:, 1:2], in1=ost[:, 0:vn],
                    op0=ALU.mult, op1=ALU.add)
                dst = bass.AP(tensor=out_t[:].tensor, offset=v0,
                              ap=[[V, 128], [1, vn]])
                # alternate ACT/Pool DMA queues: off the head-blocked sync
                # queue, and neither queue eats all the serialization
                if vb % 2 == 0:
                    nc.gpsimd.dma_start(out=dst, in_=ost[:, 0:vn])
                else:
                    nc.scalar.dma_start(out=dst, in_=ost[:, 0:vn])

    nc.compile()
    return nc


def _host_prep(inputs):
    f32 = np.float32

    def bf(x):
        return np.ascontiguousarray(np.asarray(x, f32).astype(BF))

    def q8(x):
        # fixed power-of-2 scale + clip; RNE via astype
        y = np.clip(np.asarray(x, f32) * SCL, -E3MAX, E3MAX)
        return np.ascontiguousarray(y.astype(E3))

    enc = np.asarray(inputs["encoder_out"], f32)
    enc_p = enc.reshape(B, E, PP)
    feats_all = enc.mean(axis=2)                  # (B, E, W=T)

    # permute gate blocks (i,f,g,o) -> (i,f,o,g) so one sigmoid spans i,f,o
    gp = np.r_[0:2 * D, 3 * D:4 * D, 2 * D:3 * D]

    common = {}
    common["wih1f"] = q8(np.asarray(inputs["Wih1"], f32).T[:, gp])
    common["wih1r"] = q8(np.asarray(inputs["Wih1r"], f32).T[:, gp])
    common["whh1f"] = q8(np.asarray(inputs["Whh1"], f32).T[:, gp])
    common["whh1r"] = q8(np.asarray(inputs["Whh1r"], f32).T[:, gp])
    common["wih2f"] = q8(np.asarray(inputs["Wih2"], f32).T[:, gp])
    common["wih2r"] = q8(np.asarray(inputs["Wih2r"], f32).T[:, gp])
    common["whh2f"] = q8(np.asarray(inputs["Whh2"], f32).T[:, gp])
    common["whh2r"] = q8(np.asarray(inputs["Whh2r"], f32).T[:, gp])
    common["b1f"] = bf(np.asarray(inputs["bih1"] + inputs["bhh1"],
                                  f32)[gp] * SCL)
    common["b1r"] = bf(np.asarray(inputs["bih1r"] + inputs["bhh1r"],
                                  f32)[gp] * SCL)
    common["b2f"] = bf(np.asarray(inputs["bih2"] + inputs["bhh2"],
                                  f32)[gp] * SCL)
    common["b2r"] = bf(np.asarray(inputs["bih2r"] + inputs["bhh2r"],
                                  f32)[gp] * SCL)
    common["wencT"] = q8(np.asarray(inputs["Wenc"], f32).T)
    common["wdecT"] = q8(np.asarray(inputs["Wdec"], f32).T)
    common["beab"] = bf(np.asarray(inputs["benc"] + inputs["bdec"],
                                   f32) * SCL)
    common["wfullb"] = bf(np.asarray(inputs["Wfull"], f32)[0])
    wg = np.asarray(inputs["Wg"], f32)
    common["wdiffT"] = bf(wg[0] - wg[1])
    bd = float(np.asarray(inputs["bg"], f32)[0]
               - np.asarray(inputs["bg"], f32)[1])
    common["bdiffb"] = bf(np.array([[bd]], f32))
    common["eye128"] = bf(np.eye(128, dtype=f32))
    common["wfcT"] = q8(np.asarray(inputs["Wfc"], f32).T)
    bfc = np.zeros(VCH * 128, f32)
    bfc[:V] = np.asarray(inputs["bfc"], f32)
    common["bfcp"] = bf(bfc * SCL)

    in_maps = []
    for c in range(NCORES):
        m = dict(common)
        sl = enc_p[c * NB:(c + 1) * NB]
        m["enc_ep"] = bf(sl)
        m["enc_pe"] = bf(np.ascontiguousarray(sl.transpose(0, 2, 1)))
        m["featsb"] = bf(np.ascontiguousarray(
            feats_all[c * NB:(c + 1) * NB].transpose(1, 0, 2)))
        in_maps.append(m)
    return in_maps


def _get_program():
    if "nc" not in _prog_cache:
        _prog_cache["nc"] = _build_program()
    return _prog_cache["nc"]


def kernel(**inputs):
    from concourse.bass_utils import run_bass_kernel_spmd

    nc = _get_program()
    in_maps = _host_prep(inputs)
    res = run_bass_kernel_spmd(nc, in_maps, list(range(NCORES)))
    # per-core result is [b, t, v] bf16; assemble to (T, B, V) f32
    out = np.concatenate(
        [np.asarray(res.results[c]["out"], np.float32).transpose(1, 0, 2)
         for c in range(NCORES)],
        axis=1)
    return np.ascontiguousarray(out, np.float32)
